# revision 28
# baseline (speedup 1.0000x reference)
"""Trainium2 Bass kernel for nn_DecoderLayer (B=8, S=1024, D=1024, H=16, DFF=4096).

Sharding: pure data-parallel over batch — one batch element per NeuronCore,
no collectives. Each core computes the full decoder layer for its element.

Per-core dataflow (activations kept feature-major, i.e. transposed [feat, tok]):
  qT/kT = W^T @ xT          (head-pair-major [128, 8, 1024] bf16)
  V_plus = xT^T @ Wv | ones (natural [tok, head, 64+1])
  per (pair, tok-half): logits for both heads of the pair land in one
  [128, 2, 512] PSUM tile (concurrent K=64 row-group matmuls), ONE exp
  per i-tile covers both heads; causal blocks skipped, partial blocks
  masked by 0/1 multiplier tiles.  [ctx; den] = V_plus^T @ E^T comes out
  unnormalized; ctx is evicted to SBUF bf16, den rows bounce through DRAM.
  After the last group: ONE batched ACT reciprocal over all 32 den rows
  (single table swap for the whole kernel), DMA partition-broadcast of
  the reciprocals, bf16 DVE muls normalize ctx in place.
  Wo then runs as a dense K=1024 PSUM accumulation (+bias+residual in one
  scalar_tensor_tensor), o1f kept bf16 so LN1's token sums read it
  directly; squares batched into one 3-D ACT op per token half.
  FFN1 transposed; FFN2 with swapped matmul operands -> natural [tok, feat];
  LN3 natively on PSUM (bn_stats), DMA out in natural layout.

SBUF slots are chained across phases via shared pool tags:
  xTb -> w2 resident;  qT -> h1;  kT -> o1n.
"""
import sys

sys.path.insert(0, "/opt/trn_rl_repo")

from contextlib import ExitStack

import numpy as np
import ml_dtypes

import concourse.bass as bass
import concourse.tile as tile
from concourse import mybir
from concourse.bass_utils import run_bass_kernel_spmd
from concourse.vector_clock import ScopedClock

P = 128
D = 1024
NH = 16
DEP = 64
DFF = 4096
S = 1024
NJ = D // P      # 8 feature tiles
NT = S // P      # 8 token tiles
NM = DFF // P    # 32 dff tiles
NG = NJ * 2      # 16 (pair, token-half) attention groups
EPS = 1e-6
W2RES = 8        # m-tiles of W2 kept resident in SBUF

f32 = mybir.dt.float32
bf16 = mybir.dt.bfloat16
AF = mybir.ActivationFunctionType
ALU = mybir.AluOpType

# host-side knobs (test.py may set TRACE=True for a profiled run)
TRACE = False
TRACE_DIR = None
LAST_EXEC_NS = None


class TileCtx(tile.TileContext):
    """This container's walrus rejects any instruction carrying >1 sync-wait.
    Split the kernel-tail drain's global-clock waits across single-wait
    Drains."""

    def _drain_and_barrier(self, tick_clock, wait_clock):
        nc = self.nc
        drain_inst = nc.sync.drain()
        wait_clock.add_sem_waits(
            drain_inst.ins, ScopedClock({None: tick_clock.global_clock})
        )
        waits = list(drain_inst.ins.sync_info.on_wait or [])
        if len(waits) > 1:
            del drain_inst.ins.sync_info.on_wait[1:]
            for w in waits[1:]:
                d = nc.sync.drain()
                if d.ins.sync_info is None:
                    d.ins.sync_info = mybir.SyncInfo(on_wait=[], on_update=[])
                d.ins.sync_info.on_wait.append(w)

        nc.all_engine_barrier()
        assert self.sems is not None
        popped = nc._tile_sem_poison_stack.pop()
        assert popped is self._sem_poison
        nc.clear_and_free_semaphores(list(self.sems.allocated().values()))
        nc.all_engine_barrier()


def legalize_waits(nc):
    """Split every multi-wait instruction into preceding single-wait Drains
    (same walrus limitation, applied to the whole program)."""
    import orjson

    bir = nc.to_json()
    ctr = 0
    for fn in bir["functions"]:
        for blk in fn["blocks"]:
            new = []
            for inst in blk["instructions"]:
                si = inst.get("sync_info")
                waits = (si or {}).get("on_wait") or []
                if len(waits) > 1:
                    for w in waits[:-1]:
                        ctr += 1
                        new.append({
                            "engine": inst["engine"],
                            "ins": [], "outs": [],
                            "name": f"I-wfix{ctr}",
                            "opcode": "NoOp",
                            "sync_info": {"on_update": [], "on_wait": [w]},
                            "debug": inst.get("debug"),
                        })
                    si["on_wait"] = [waits[-1]]
                new.append(inst)
            blk["instructions"] = new
    blob = orjson.dumps(bir)
    nc.to_json_bytes = lambda: blob
    return ctr


def _dve_recip(nc, out, in_):
    """DVE reciprocal with a low-precision (bf16) output; bass's wrapper
    fatals on bf16 out, but softmax denominators only need ~bf16 accuracy."""
    eng = nc.vector
    return eng.add_instruction(
        mybir.InstReciprocal(
            name=nc.get_next_instruction_name(),
            ins=[eng.lower_ap(in_)],
            outs=[eng.lower_ap(out)],
        )
    )


def _rep2(ap2d):
    """[128, N] AP -> [128, 2, N] AP with a stride-0 middle dim (same data
    fed to both heads of a pair)."""
    return bass.AP(
        tensor=ap2d.tensor, offset=ap2d.offset,
        ap=[list(ap2d.ap[0])] + [[0, 2]] + [list(p) for p in ap2d.ap[1:]],
    )


def _block_plan(mask_qk):
    """Classify [128 x 128] blocks of the visibility pattern.

    mask_qk: (S, S) bool, True where key k is VISIBLE to query q ([q, k]).
    Returns status[i][j] for sk-tile i, sq-tile j ('full'|'part'|'skip')
    and is_causal (enables narrow per-row column ranges).
    """
    vis_T = mask_qk.T  # [k, q]
    status = [[None] * NT for _ in range(NT)]
    for i in range(NT):
        for j in range(NT):
            blk = vis_T[i * P:(i + 1) * P, j * P:(j + 1) * P]
            status[i][j] = "full" if blk.all() else ("skip" if not blk.any()
                                                    else "part")
    causal = np.tril(np.ones((S, S), dtype=bool))
    return status, bool((mask_qk == causal).all())


def build_nc(status, is_causal):
    nc = bass.Bass()

    # ---- I/O -------------------------------------------------------------
    xTb_d = nc.declare_dram_parameter("xTb", [D, S], bf16, isOutput=False)
    wq_d = nc.declare_dram_parameter("wq", [D, D], bf16, isOutput=False)
    wk_d = nc.declare_dram_parameter("wk", [D, D], bf16, isOutput=False)
    wv_d = nc.declare_dram_parameter("wv", [D, D], bf16, isOutput=False)
    wo_d = nc.declare_dram_parameter("wo", [D, D], bf16, isOutput=False)
    w1_d = nc.declare_dram_parameter("w1", [D, DFF], bf16, isOutput=False)
    w2_d = nc.declare_dram_parameter("w2", [DFF, D], bf16, isOutput=False)
    bq_d = nc.declare_dram_parameter("bq", [D], f32, isOutput=False)
    bk_d = nc.declare_dram_parameter("bk", [D], f32, isOutput=False)
    bv_d = nc.declare_dram_parameter("bv", [D], f32, isOutput=False)
    bo_d = nc.declare_dram_parameter("bo", [D], f32, isOutput=False)
    b1_d = nc.declare_dram_parameter("b1", [DFF], f32, isOutput=False)
    b2_d = nc.declare_dram_parameter("b2", [D], f32, isOutput=False)
    g1_d = nc.declare_dram_parameter("g1", [D], f32, isOutput=False)
    be1_d = nc.declare_dram_parameter("be1", [D], f32, isOutput=False)
    g3_d = nc.declare_dram_parameter("g3", [D], f32, isOutput=False)
    be3_d = nc.declare_dram_parameter("be3", [D], f32, isOutput=False)
    maskm_d = nc.declare_dram_parameter("maskm", [S, S], bf16, isOutput=False)
    out_d = nc.declare_dram_parameter("out", [S, D], bf16, isOutput=True)

    # DRAM scratch for the per-group softmax-denominator partition broadcast
    den_d = nc.dram_tensor("den_sc", [NG, 2, 512], bf16, kind="Internal")

    xTb_v = xTb_d[:, :].rearrange("(ko ki) t -> ki ko t", ki=P)
    wq_v = wq_d[:, :].rearrange("(ko ki) n -> ki ko n", ki=P)
    wk_v = wk_d[:, :].rearrange("(ko ki) n -> ki ko n", ki=P)
    wv_v = wv_d[:, :].rearrange("(ko ki) n -> ki ko n", ki=P)
    wo_v = wo_d[:, :].rearrange("(ko ki) n -> ki ko n", ki=P)
    w1_v = w1_d[:, :].rearrange("(ko ki) n -> ki ko n", ki=P)
    w2_v = w2_d[:, :].rearrange("(mo ki) n -> ki mo n", ki=P)

    def bcast_ap(src_1d, parts):
        """1-D DRAM AP [N] -> stride-0 partition-broadcast AP [parts, N]."""
        return bass.AP(
            tensor=src_1d.tensor, offset=src_1d.offset,
            ap=[[0, parts]] + [list(p) for p in src_1d.ap],
        )

    def bcast2_ap(src_2d, reps):
        """2-D DRAM AP [2, N] -> [2, reps, N] AP (each row replicated)."""
        return bass.AP(
            tensor=src_2d.tensor, offset=src_2d.offset,
            ap=[list(src_2d.ap[0])] + [[0, reps]] + [list(src_2d.ap[1])],
        )

    with TileCtx(nc) as tc, ExitStack() as ctx:
        sing = ctx.enter_context(tc.tile_pool(name="sing", bufs=1))
        bigp = ctx.enter_context(tc.tile_pool(name="bigp", bufs=1))
        wpool = ctx.enter_context(tc.tile_pool(name="wpool", bufs=2))
        rot = ctx.enter_context(tc.tile_pool(name="rot", bufs=2))
        statp = ctx.enter_context(tc.tile_pool(name="statp", bufs=2))
        epool = ctx.enter_context(tc.tile_pool(name="epool", bufs=3))
        bcpool = ctx.enter_context(tc.tile_pool(name="bcpool", bufs=3))
        w2pool = ctx.enter_context(tc.tile_pool(name="w2pool", bufs=5))
        onatp = ctx.enter_context(tc.tile_pool(name="onatp", bufs=2))
        proj_ctx = ExitStack()
        ps_mm = proj_ctx.enter_context(
            tc.tile_pool(name="ps_mm", bufs=2, space="PSUM"))

        # ---- constants / params ------------------------------------------
        # slot chain "bigx": xTb (2MB) -> w2 resident half (2MB)
        xTb = bigp.tile([P, NJ, S], bf16, tag="bigx")
        for j in range(NJ):
            eng = nc.sync if j % 2 == 0 else nc.gpsimd
            eng.dma_start(xTb[:, j, :], xTb_v[:, j, :])

        def load_bias_T(d_ap, ko, tag):
            t = sing.tile([P, ko], f32, tag=tag)
            nc.gpsimd.dma_start(t, d_ap[:].rearrange("(ko ki) -> ki ko", ki=P))
            return t

        bqT = load_bias_T(bq_d, NJ, "bqT")
        bkT = load_bias_T(bk_d, NJ, "bkT")
        boT = load_bias_T(bo_d, NJ, "boT")
        b1T = load_bias_T(b1_d, NM, "b1T")
        g1T = load_bias_T(g1_d, NJ, "g1T")
        be1T = load_bias_T(be1_d, NJ, "be1T")

        bv_b = sing.tile([P, D], f32, tag="natb")
        nc.gpsimd.dma_start(bv_b, bcast_ap(bv_d[:], P))
        ones_bf = sing.tile([P, P], bf16, tag="ones_bf")
        nc.vector.memset(ones_bf, 1.0)
        eps_t = sing.tile([P, 1], f32, tag="eps_t")
        nc.vector.memset(eps_t, EPS)

        # ---- Q/K projections: [128, 8(pair), 1024] bf16 ------------------
        # slot chain "o1h": qT (2MB) -> h1 x2 (4MB)
        qT = bigp.tile([P, NJ, S], bf16, tag="o1h")
        kT = sing.tile([P, NJ, S], bf16, tag="ko")
        for w_v, out_sb, bias_sb in ((wq_v, qT, bqT), (wk_v, kT, bkT)):
            for nb in range(2):
                wt = wpool.tile([P, NJ, 512], bf16, tag="wbig")
                for j in range(NJ):
                    eng = nc.sync if j % 2 == 0 else nc.gpsimd
                    eng.dma_start(
                        wt[:, j, :], w_v[:, j, nb * 512:(nb + 1) * 512])
                for nn in range(4):
                    n = nb * 4 + nn
                    for Hh in range(2):
                        psum = ps_mm.tile([P, 512], f32, tag="mm")
                        for j in range(NJ):
                            nc.tensor.matmul(
                                psum,
                                wt[:, j, nn * P:(nn + 1) * P],
                                xTb[:, j, Hh * 512:(Hh + 1) * 512],
                                start=(j == 0), stop=(j == NJ - 1),
                            )
                        osl = out_sb[:, n, Hh * 512:(Hh + 1) * 512]
                        if out_sb is kT:
                            nc.vector.tensor_scalar_add(
                                osl, psum, bias_sb[:, n:n + 1])
                        else:
                            nc.scalar.activation(
                                osl, psum, AF.Identity,
                                bias=bias_sb[:, n:n + 1])

        mtiles = {}
        if is_causal:
            dmt = sing.tile([P, P], bf16, tag="dmt")
            nc.gpsimd.dma_start(dmt, maskm_d[0:P, 0:P])
            for i in range(NT):
                mtiles[(i, i)] = dmt
        else:
            for i in range(NT):
                for j in range(NT):
                    if status[i][j] == "part":
                        t = sing.tile([P, P], bf16, tag=f"mt{i}_{j}",
                                      name=f"mt{i}_{j}")
                        nc.sync.dma_start(
                            t, maskm_d[i * P:(i + 1) * P, j * P:(j + 1) * P]
                        )
                        mtiles[(i, j)] = t

        # ---- V projection -> V_plus [128, 16, 65] per token tile ---------
        vps = []
        for i in range(NT):
            vp = sing.tile([P, NH, DEP + 1], bf16, tag=f"vp{i}", name=f"vp{i}")
            nc.vector.memset(vp[:, :, DEP:DEP + 1], 1.0)
            vps.append(vp)
        for dh in range(2):
            wt = wpool.tile([P, NJ, 512], bf16, tag="wbig")
            nc.sync.dma_start(wt, wv_v[:, :, dh * 512:(dh + 1) * 512])
            for i in range(NT):
                psum = ps_mm.tile([P, 512], f32, tag="mm")
                for j in range(NJ):
                    nc.tensor.matmul(
                        psum,
                        xTb[:, j, i * P:(i + 1) * P],
                        wt[:, j, :],
                        start=(j == 0), stop=(j == NJ - 1),
                    )
                nc.vector.tensor_add(
                    out=vps[i][:, dh * 8:(dh + 1) * 8, 0:DEP],
                    in0=psum[:, :].rearrange("p (l d) -> p l d", d=DEP),
                    in1=bv_b[:, dh * 512:(dh + 1) * 512].rearrange(
                        "p (l d) -> p l d", d=DEP),
                )

        # Wo resident for the post-attention dense accumulation
        wo_sb = sing.tile([P, NJ, D], bf16, tag="wo_sb")
        nc.gpsimd.dma_start(wo_sb, wo_v[:, :, :])

        # free the projection PSUM banks so attention can double-buffer
        proj_ctx.close()

        # ---- attention ----------------------------------------------------
        # unnormalized ctx, feature-major [dep+sub | pair | tok], bf16
        ctxT = sing.tile([P, NJ, S], bf16, tag="ctxT")

        attn_ctx = ExitStack()
        ps_lg = attn_ctx.enter_context(
            tc.tile_pool(name="ps_lg", bufs=2, space="PSUM"))
        ps_ctx = attn_ctx.enter_context(
            tc.tile_pool(name="ps_ctx", bufs=2, space="PSUM"))
        for pair in range(NJ):
            for Hh in range(2):
                if is_causal:
                    i_list = [i for i in range(NT)
                              if any(status[i][j] != "skip"
                                     for j in range(Hh * 4, Hh * 4 + 4))]
                else:
                    i_list = list(range(NT))
                psc = ps_ctx.tile([P, 2, 512], f32, tag="ctx", name="psc")
                for idx, i in enumerate(i_list):
                    s0 = max(0, i * P - Hh * 512) if is_causal else 0
                    W = 512 - s0
                    # the two heads of the pair sit in array row-groups
                    # 0-63 / 64-127 -> adjacent K=64 matmuls run concurrently;
                    # both land in one 2-bank PSUM tile for a single exp
                    plg = ps_lg.tile([P, 2, 512], f32, tag="lg")
                    for sub in range(2):
                        pb = sub * DEP
                        nc.tensor.matmul(
                            plg[:, sub, s0:512],
                            kT[pb:pb + DEP, pair, i * P:(i + 1) * P],
                            qT[pb:pb + DEP, pair,
                               Hh * 512 + s0:(Hh + 1) * 512],
                            start=True, stop=True,
                        )
                    et = epool.tile([P, 2, 512], bf16, tag="e")
                    nc.scalar.activation(
                        et[:, :, s0:512], plg[:, :, s0:512], AF.Exp,
                        scale=0.125,
                    )
                    for j in range((Hh * 512 + s0) // P, Hh * 4 + 4):
                        if status[i][j] == "full":
                            continue
                        c = j * P - Hh * 512
                        mt = mtiles.get((i, j))
                        if mt is None:  # 'skip' inside computed range
                            nc.vector.memset(et[:, :, c:c + P], 0.0)
                        else:
                            nc.vector.tensor_mul(
                                et[:, :, c:c + P], et[:, :, c:c + P],
                                _rep2(mt),
                            )
                    for sub in range(2):
                        h = pair * 2 + sub
                        nc.tensor.matmul(
                            psc[0:DEP + 1, sub, s0:512],
                            vps[i][:, h, :],
                            et[:, sub, s0:512],
                            start=(idx == 0), stop=(idx == len(i_list) - 1),
                        )
                # evict unnormalized ctx (cross-partition bf16 copies),
                # reciprocal both den rows on DVE, bounce them through DRAM
                # for the partition broadcast, and normalize ctx in place —
                # all per-group, overlapping the next group's exps
                g = pair * 2 + Hh
                cols = slice(Hh * 512, (Hh + 1) * 512)
                nc.vector.tensor_copy(ctxT[0:DEP, pair, cols],
                                      psc[0:DEP, 0, :])
                nc.vector.tensor_copy(ctxT[DEP:P, pair, cols],
                                      psc[0:DEP, 1, :])
                dent = bcpool.tile([1, 2, 512], bf16, tag="dent", bufs=2)
                _dve_recip(nc, dent, psc[DEP:DEP + 1, :, :])
                nc.gpsimd.dma_start(den_d[g, :, :], dent)
                bcs = bcpool.tile([P, 512], bf16, tag="bc", bufs=2)
                nc.sync.dma_start(bcs, bcast2_ap(den_d[g, :, :], DEP))
                nc.vector.tensor_mul(
                    ctxT[:, pair, cols], ctxT[:, pair, cols], bcs)

        attn_ctx.close()
        ps_mm = ctx.enter_context(
            tc.tile_pool(name="ps_mm2", bufs=2, space="PSUM"))

        # ---- Wo + bias + residual -> o1f (bf16), then LN1 -----------------
        o1f = bigp.tile([P, NJ, S], bf16, tag="o1f")
        for n in range(NJ):
            for Hh in range(2):
                cols = slice(Hh * 512, (Hh + 1) * 512)
                pw = ps_mm.tile([P, 512], f32, tag="mm")
                for pair in range(NJ):
                    nc.tensor.matmul(
                        pw,
                        wo_sb[:, pair, n * P:(n + 1) * P],
                        ctxT[:, pair, cols],
                        start=(pair == 0), stop=(pair == NJ - 1),
                    )
                nc.vector.scalar_tensor_tensor(
                    out=o1f[:, n, cols], in0=pw, scalar=boT[:, n:n + 1],
                    in1=xTb[:, n, cols], op0=ALU.add, op1=ALU.add,
                )

        # ---- LN1 (transposed; sums read bf16 o1f directly) ----------------
        ln_ctx = ExitStack()
        ps_ln = ln_ctx.enter_context(
            tc.tile_pool(name="ps_ln", bufs=1, space="PSUM"))
        ps_s = [ps_ln.tile([P, 512], f32, tag=f"lns{Hh}", name=f"lns{Hh}")
                for Hh in range(2)]
        ps_q = [ps_ln.tile([P, 512], f32, tag=f"lnq{Hh}", name=f"lnq{Hh}")
                for Hh in range(2)]
        o1n = sing.tile([P, NJ, S], bf16, tag="ko")
        for Hh in range(2):
            cols = slice(Hh * 512, (Hh + 1) * 512)
            sq = rot.tile([P, NJ, 512], bf16, tag="sq", bufs=1)
            nc.scalar.activation(sq, o1f[:, :, cols], AF.Square)
            for n in range(NJ):
                nc.tensor.matmul(ps_s[Hh], ones_bf, o1f[:, n, cols],
                                 start=(n == 0), stop=(n == NJ - 1))
                nc.tensor.matmul(ps_q[Hh], ones_bf, sq[:, n, :],
                                 start=(n == 0), stop=(n == NJ - 1))
            mean = statp.tile([P, 512], f32, tag="mean")
            nc.vector.tensor_scalar_mul(mean, ps_s[Hh], 1.0 / D)
            m2 = rot.tile([P, 512], f32, tag="sub")
            nc.vector.tensor_mul(m2, mean, mean)
            var = statp.tile([P, 512], f32, tag="var")
            nc.vector.scalar_tensor_tensor(
                out=var, in0=ps_q[Hh], scalar=1.0 / D, in1=m2,
                op0=ALU.mult, op1=ALU.subtract,
            )
            nc.scalar.activation(var, var, AF.Sqrt, bias=eps_t)
            nc.vector.reciprocal(var, var)  # rstd
            for j in range(NJ):
                sl = o1f[:, j, cols]
                sub = rot.tile([P, 512], f32, tag="sub")
                nc.vector.tensor_sub(sub, sl, mean)
                nc.vector.tensor_mul(sub, sub, var)
                nc.vector.tensor_scalar(
                    out=o1n[:, j, cols], in0=sub,
                    scalar1=g1T[:, j:j + 1], scalar2=be1T[:, j:j + 1],
                    op0=ALU.mult, op1=ALU.add,
                )

        # ---- FFN + LN3 (FFN2 swapped -> natural layout) -------------------
        ln_ctx.close()

        b2_b = sing.tile([P, D], f32, tag="natb")
        nc.gpsimd.dma_start(b2_b, bcast_ap(b2_d[:], P))
        g3_b = sing.tile([P, D], f32, tag="g3_b")
        nc.gpsimd.dma_start(g3_b, bcast_ap(g3_d[:], P))
        be3_b = sing.tile([P, D], f32, tag="be3_b")
        nc.gpsimd.dma_start(be3_b, bcast_ap(be3_d[:], P))
        nat_ctx = ExitStack()
        ps_nat = nat_ctx.enter_context(
            tc.tile_pool(name="ps_nat", bufs=2, space="PSUM"))
        w2h = bigp.tile([P, W2RES, D], bf16, tag="bigx")
        nc.sync.dma_start(w2h, w2_v[:, :W2RES, :])
        for Hh in range(2):
            h1 = bigp.tile([P, NM, 512], bf16, tag="o1h")
            for nb in range(8):
                wt = wpool.tile([P, NJ, 512], bf16, tag="wbig")
                nc.sync.dma_start(wt, w1_v[:, :, nb * 512:(nb + 1) * 512])
                for mloc in range(4):
                    m = nb * 4 + mloc
                    psum = ps_mm.tile([P, 512], f32, tag="mm")
                    for j in range(NJ):
                        nc.tensor.matmul(
                            psum,
                            wt[:, j, mloc * P:(mloc + 1) * P],
                            o1n[:, j, Hh * 512:(Hh + 1) * 512],
                            start=(j == 0), stop=(j == NJ - 1),
                        )
                    nc.scalar.activation(
                        h1[:, m, :], psum, AF.Relu, bias=b1T[:, m:m + 1]
                    )
            for tp in range(2):
                pnats = [ps_nat.tile([P, D], f32, tag="nat", name=f"nat{Hh}{tp}{ti}")
                         for ti in range(2)]
                for m in range(NM):
                    if m < W2RES:
                        w2t = w2h[:, m, :]
                    else:
                        w2t = w2pool.tile([P, D], bf16, tag="w2t")
                        nc.gpsimd.dma_start(w2t, w2_v[:, m, :])
                    for ti in range(2):
                        tloc = tp * 2 + ti
                        for half in range(2):
                            nc.tensor.matmul(
                                pnats[ti][:, half * 512:(half + 1) * 512],
                                h1[:, m, tloc * P:(tloc + 1) * P],
                                w2t[:, half * 512:(half + 1) * 512],
                                start=(m == 0), stop=(m == NM - 1),
                            )
                for ti in range(2):
                    t = Hh * 4 + tp * 2 + ti
                    pnat = pnats[ti]
                    onat = onatp.tile([P, D], f32, tag="onat")
                    nc.scalar.activation(onat, pnat, AF.Copy)  # frees psum
                    nc.vector.tensor_add(onat, onat, b2_b)
                    stats = statp.tile([P, 2, 6], f32, tag="bnst")
                    nc.vector.bn_stats(stats[:, 0, :], onat[:, 0:512])
                    nc.vector.bn_stats(stats[:, 1, :], onat[:, 512:1024])
                    mv = statp.tile([P, 2], f32, tag="bnmv")
                    nc.vector.bn_aggr(mv, stats)
                    rs = statp.tile([P, 1], f32, tag="bnrs")
                    nc.scalar.activation(rs, mv[:, 1:2], AF.Sqrt, bias=eps_t)
                    nc.vector.reciprocal(rs, rs)
                    nc.vector.tensor_scalar(
                        out=onat, in0=onat, scalar1=mv[:, 0:1], scalar2=rs,
                        op0=ALU.subtract, op1=ALU.mult,
                    )
                    nc.vector.tensor_mul(onat, onat, g3_b)
                    obf = rot.tile([P, D], bf16, tag="sub")
                    nc.vector.tensor_add(obf, onat, be3_b)
                    nc.sync.dma_start(out_d[t * P:(t + 1) * P, :], obf)
        nat_ctx.close()

    return nc


_BUILD_CACHE = {}


def _get_nc(mask_qk):
    key = mask_qk.tobytes()
    if key not in _BUILD_CACHE:
        status, is_causal = _block_plan(mask_qk)
        nc = build_nc(status, is_causal)
        legalize_waits(nc)
        _BUILD_CACHE[key] = nc
    return _BUILD_CACHE[key]


def kernel(x, look_ahead_mask, wq, bq, wk, bk, wv, bv, wo, bo,
           w1, b1, w2, b2, ln1_g, ln1_b, ln3_g, ln3_b):
    global LAST_EXEC_NS
    x = np.asarray(x, dtype=np.float32)
    B = x.shape[0]
    mask = np.asarray(look_ahead_mask, dtype=np.float32)[0, 0]
    mask_qk = mask == 0.0  # True where key visible to query
    maskm = np.ascontiguousarray(mask_qk.T).astype(ml_dtypes.bfloat16)

    nc = _get_nc(mask_qk)

    bf = ml_dtypes.bfloat16
    shared = {
        "wq": np.ascontiguousarray(wq).astype(bf),
        "wk": np.ascontiguousarray(wk).astype(bf),
        "wv": np.ascontiguousarray(wv).astype(bf),
        "wo": np.ascontiguousarray(wo).astype(bf),
        "w1": np.ascontiguousarray(w1).astype(bf),
        "w2": np.ascontiguousarray(w2).astype(bf),
        "bq": np.asarray(bq, np.float32), "bk": np.asarray(bk, np.float32),
        "bv": np.asarray(bv, np.float32), "bo": np.asarray(bo, np.float32),
        "b1": np.asarray(b1, np.float32), "b2": np.asarray(b2, np.float32),
        "g1": np.asarray(ln1_g, np.float32),
        "be1": np.asarray(ln1_b, np.float32),
        "g3": np.asarray(ln3_g, np.float32),
        "be3": np.asarray(ln3_b, np.float32),
        "maskm": maskm,
    }
    in_maps = []
    for c in range(8):
        m = dict(shared)
        m["xTb"] = np.ascontiguousarray(x[c % B].T).astype(bf)
        in_maps.append(m)

    kwargs = {}
    if TRACE:
        kwargs = {"trace": True, "tmpdir": TRACE_DIR}
    res = run_bass_kernel_spmd(nc, in_maps, list(range(8)), **kwargs)
    LAST_EXEC_NS = res.exec_time_ns
    out = np.stack([res.results[c]["out"] for c in range(B)], axis=0)
    return out.astype(np.float32)


# revision 30
# speedup vs baseline: 1.1102x; 1.1102x over previous
"""Trainium2 Bass kernel for nn_DecoderLayer (B=8, S=1024, D=1024, H=16, DFF=4096).

Sharding: pure data-parallel over batch — one batch element per NeuronCore,
no collectives. Each core computes the full decoder layer for its element.

Per-core dataflow (activations kept feature-major, i.e. transposed [feat, tok]):
  qT/kT = W^T @ xT          (head-pair-major [128, 8, 1024] bf16)
  V_plus = xT^T @ Wv | ones (natural [tok, head, 64+1])
  per (pair, tok-half): logits for both heads of the pair land in one
  [128, 2, 512] PSUM tile (concurrent K=64 row-group matmuls), ONE exp
  per i-tile covers both heads; causal blocks skipped, partial blocks
  masked by 0/1 multiplier tiles.  [ctx; den] = V_plus^T @ E^T comes out
  unnormalized; ctx is evicted to SBUF bf16, den rows bounce through DRAM.
  After the last group: ONE batched ACT reciprocal over all 32 den rows
  (single table swap for the whole kernel), DMA partition-broadcast of
  the reciprocals, bf16 DVE muls normalize ctx in place.
  Wo then runs as a dense K=1024 PSUM accumulation (+bias+residual in one
  scalar_tensor_tensor), o1f kept bf16 so LN1's token sums read it
  directly; squares batched into one 3-D ACT op per token half.
  FFN1 transposed; FFN2 with swapped matmul operands -> natural [tok, feat];
  LN3 natively on PSUM (bn_stats), DMA out in natural layout.

SBUF slots are chained across phases via shared pool tags:
  xTb -> w2 resident;  qT -> h1;  kT -> o1n.
"""
import sys

sys.path.insert(0, "/opt/trn_rl_repo")

from contextlib import ExitStack

import numpy as np
import ml_dtypes

import concourse.bass as bass
import concourse.tile as tile
from concourse import mybir
from concourse.bass_utils import run_bass_kernel_spmd
from concourse.vector_clock import ScopedClock

P = 128
D = 1024
NH = 16
DEP = 64
DFF = 4096
S = 1024
NJ = D // P      # 8 feature tiles
NT = S // P      # 8 token tiles
NM = DFF // P    # 32 dff tiles
NG = NJ * 2      # 16 (pair, token-half) attention groups
EPS = 1e-6
W2RES = 8        # m-tiles of W2 kept resident in SBUF

f32 = mybir.dt.float32
bf16 = mybir.dt.bfloat16
AF = mybir.ActivationFunctionType
ALU = mybir.AluOpType

# host-side knobs (test.py may set TRACE=True for a profiled run)
TRACE = False
TRACE_DIR = None
LAST_EXEC_NS = None


class TileCtx(tile.TileContext):
    """This container's walrus rejects any instruction carrying >1 sync-wait.
    Split the kernel-tail drain's global-clock waits across single-wait
    Drains."""

    def _drain_and_barrier(self, tick_clock, wait_clock):
        nc = self.nc
        drain_inst = nc.sync.drain()
        wait_clock.add_sem_waits(
            drain_inst.ins, ScopedClock({None: tick_clock.global_clock})
        )
        waits = list(drain_inst.ins.sync_info.on_wait or [])
        if len(waits) > 1:
            del drain_inst.ins.sync_info.on_wait[1:]
            for w in waits[1:]:
                d = nc.sync.drain()
                if d.ins.sync_info is None:
                    d.ins.sync_info = mybir.SyncInfo(on_wait=[], on_update=[])
                d.ins.sync_info.on_wait.append(w)

        nc.all_engine_barrier()
        assert self.sems is not None
        popped = nc._tile_sem_poison_stack.pop()
        assert popped is self._sem_poison
        nc.clear_and_free_semaphores(list(self.sems.allocated().values()))
        nc.all_engine_barrier()


def legalize_waits(nc):
    """Split every multi-wait instruction into preceding single-wait Drains
    (same walrus limitation, applied to the whole program)."""
    import orjson

    bir = nc.to_json()
    ctr = 0
    for fn in bir["functions"]:
        for blk in fn["blocks"]:
            new = []
            for inst in blk["instructions"]:
                si = inst.get("sync_info")
                waits = (si or {}).get("on_wait") or []
                if len(waits) > 1:
                    for w in waits[:-1]:
                        ctr += 1
                        new.append({
                            "engine": inst["engine"],
                            "ins": [], "outs": [],
                            "name": f"I-wfix{ctr}",
                            "opcode": "NoOp",
                            "sync_info": {"on_update": [], "on_wait": [w]},
                            "debug": inst.get("debug"),
                        })
                    si["on_wait"] = [waits[-1]]
                new.append(inst)
            blk["instructions"] = new
    blob = orjson.dumps(bir)
    nc.to_json_bytes = lambda: blob
    return ctr


def _dve_recip(nc, out, in_):
    """DVE reciprocal with a low-precision (bf16) output; bass's wrapper
    fatals on bf16 out, but softmax denominators only need ~bf16 accuracy."""
    eng = nc.vector
    return eng.add_instruction(
        mybir.InstReciprocal(
            name=nc.get_next_instruction_name(),
            ins=[eng.lower_ap(in_)],
            outs=[eng.lower_ap(out)],
        )
    )


def _rep2(ap2d):
    """[128, N] AP -> [128, 2, N] AP with a stride-0 middle dim (same data
    fed to both heads of a pair)."""
    return bass.AP(
        tensor=ap2d.tensor, offset=ap2d.offset,
        ap=[list(ap2d.ap[0])] + [[0, 2]] + [list(p) for p in ap2d.ap[1:]],
    )


def _block_plan(mask_qk):
    """Classify [128 x 128] blocks of the visibility pattern.

    mask_qk: (S, S) bool, True where key k is VISIBLE to query q ([q, k]).
    Returns status[i][j] for sk-tile i, sq-tile j ('full'|'part'|'skip')
    and is_causal (enables narrow per-row column ranges).
    """
    vis_T = mask_qk.T  # [k, q]
    status = [[None] * NT for _ in range(NT)]
    for i in range(NT):
        for j in range(NT):
            blk = vis_T[i * P:(i + 1) * P, j * P:(j + 1) * P]
            status[i][j] = "full" if blk.all() else ("skip" if not blk.any()
                                                    else "part")
    causal = np.tril(np.ones((S, S), dtype=bool))
    return status, bool((mask_qk == causal).all())


def build_nc(status, is_causal):
    nc = bass.Bass()

    # ---- I/O -------------------------------------------------------------
    xTb_d = nc.declare_dram_parameter("xTb", [D, S], bf16, isOutput=False)
    wq_d = nc.declare_dram_parameter("wq", [D, D], bf16, isOutput=False)
    wk_d = nc.declare_dram_parameter("wk", [D, D], bf16, isOutput=False)
    wv_d = nc.declare_dram_parameter("wv", [D, D], bf16, isOutput=False)
    wo_d = nc.declare_dram_parameter("wo", [D, D], bf16, isOutput=False)
    w1_d = nc.declare_dram_parameter("w1", [D, DFF], bf16, isOutput=False)
    w2_d = nc.declare_dram_parameter("w2", [DFF, D], bf16, isOutput=False)
    bq_d = nc.declare_dram_parameter("bq", [D], f32, isOutput=False)
    bk_d = nc.declare_dram_parameter("bk", [D], f32, isOutput=False)
    bv_d = nc.declare_dram_parameter("bv", [D], f32, isOutput=False)
    bo_d = nc.declare_dram_parameter("bo", [D], f32, isOutput=False)
    b1_d = nc.declare_dram_parameter("b1", [DFF], f32, isOutput=False)
    b2_d = nc.declare_dram_parameter("b2", [D], f32, isOutput=False)
    g1_d = nc.declare_dram_parameter("g1", [D], f32, isOutput=False)
    be1_d = nc.declare_dram_parameter("be1", [D], f32, isOutput=False)
    g3_d = nc.declare_dram_parameter("g3", [D], f32, isOutput=False)
    be3_d = nc.declare_dram_parameter("be3", [D], f32, isOutput=False)
    maskm_d = nc.declare_dram_parameter("maskm", [S, S], bf16, isOutput=False)
    out_d = nc.declare_dram_parameter("out", [S, D], bf16, isOutput=True)

    # DRAM scratch for the softmax-denominator partition broadcast: raw den
    # rows bounce out per group, reciprocals bounce back per 8-group batch
    den_d = nc.dram_tensor("den_sc", [NG, 2, 512], bf16, kind="Internal")
    rec_d = nc.dram_tensor("rec_sc", [NG, 2, 512], bf16, kind="Internal")

    xTb_v = xTb_d[:, :].rearrange("(ko ki) t -> ki ko t", ki=P)
    wq_v = wq_d[:, :].rearrange("(ko ki) n -> ki ko n", ki=P)
    wk_v = wk_d[:, :].rearrange("(ko ki) n -> ki ko n", ki=P)
    wv_v = wv_d[:, :].rearrange("(ko ki) n -> ki ko n", ki=P)
    wo_v = wo_d[:, :].rearrange("(ko ki) n -> ki ko n", ki=P)
    w1_v = w1_d[:, :].rearrange("(ko ki) n -> ki ko n", ki=P)
    w2_v = w2_d[:, :].rearrange("(mo ki) n -> ki mo n", ki=P)

    def bcast_ap(src_1d, parts):
        """1-D DRAM AP [N] -> stride-0 partition-broadcast AP [parts, N]."""
        return bass.AP(
            tensor=src_1d.tensor, offset=src_1d.offset,
            ap=[[0, parts]] + [list(p) for p in src_1d.ap],
        )

    def bcast2_ap(src_2d, reps):
        """2-D DRAM AP [2, N] -> [2, reps, N] AP (each row replicated)."""
        return bass.AP(
            tensor=src_2d.tensor, offset=src_2d.offset,
            ap=[list(src_2d.ap[0])] + [[0, reps]] + [list(src_2d.ap[1])],
        )

    with TileCtx(nc) as tc, ExitStack() as ctx:
        sing = ctx.enter_context(tc.tile_pool(name="sing", bufs=1))
        bigp = ctx.enter_context(tc.tile_pool(name="bigp", bufs=1))
        wpool = ctx.enter_context(tc.tile_pool(name="wpool", bufs=2))
        rot = ctx.enter_context(tc.tile_pool(name="rot", bufs=2))
        statp = ctx.enter_context(tc.tile_pool(name="statp", bufs=2))
        epool = ctx.enter_context(tc.tile_pool(name="epool", bufs=3))
        bcpool = ctx.enter_context(tc.tile_pool(name="bcpool", bufs=3))
        w2pool = ctx.enter_context(tc.tile_pool(name="w2pool", bufs=5))
        onatp = ctx.enter_context(tc.tile_pool(name="onatp", bufs=2))
        proj_ctx = ExitStack()
        ps_mm = proj_ctx.enter_context(
            tc.tile_pool(name="ps_mm", bufs=2, space="PSUM"))

        # ---- constants / params ------------------------------------------
        # slot chain "bigx": xTb (2MB) -> w2 resident half (2MB)
        xTb = bigp.tile([P, NJ, S], bf16, tag="bigx")
        for j in range(NJ):
            eng = nc.sync if j % 2 == 0 else nc.gpsimd
            eng.dma_start(xTb[:, j, :], xTb_v[:, j, :])

        def load_bias_T(d_ap, ko, tag):
            t = sing.tile([P, ko], f32, tag=tag)
            nc.gpsimd.dma_start(t, d_ap[:].rearrange("(ko ki) -> ki ko", ki=P))
            return t

        bqT = load_bias_T(bq_d, NJ, "bqT")
        bkT = load_bias_T(bk_d, NJ, "bkT")
        boT = load_bias_T(bo_d, NJ, "boT")
        b1T = load_bias_T(b1_d, NM, "b1T")
        g1T = load_bias_T(g1_d, NJ, "g1T")
        be1T = load_bias_T(be1_d, NJ, "be1T")

        bv_b = sing.tile([P, D], f32, tag="natb")
        nc.gpsimd.dma_start(bv_b, bcast_ap(bv_d[:], P))
        ones_bf = sing.tile([P, P], bf16, tag="ones_bf")
        nc.vector.memset(ones_bf, 1.0)
        eps_t = sing.tile([P, 1], f32, tag="eps_t")
        nc.vector.memset(eps_t, EPS)

        # ---- Q/K projections: [128, 8(pair), 1024] bf16 ------------------
        # slot chain "o1h": qT (2MB) -> h1 x2 (4MB)
        qT = bigp.tile([P, NJ, S], bf16, tag="o1h")
        kT = sing.tile([P, NJ, S], bf16, tag="ko")
        for w_v, out_sb, bias_sb in ((wq_v, qT, bqT), (wk_v, kT, bkT)):
            for nb in range(2):
                wt = wpool.tile([P, NJ, 512], bf16, tag="wbig")
                for j in range(NJ):
                    eng = nc.sync if j % 2 == 0 else nc.gpsimd
                    eng.dma_start(
                        wt[:, j, :], w_v[:, j, nb * 512:(nb + 1) * 512])
                for nn in range(4):
                    n = nb * 4 + nn
                    for Hh in range(2):
                        psum = ps_mm.tile([P, 512], f32, tag="mm")
                        for j in range(NJ):
                            nc.tensor.matmul(
                                psum,
                                wt[:, j, nn * P:(nn + 1) * P],
                                xTb[:, j, Hh * 512:(Hh + 1) * 512],
                                start=(j == 0), stop=(j == NJ - 1),
                            )
                        osl = out_sb[:, n, Hh * 512:(Hh + 1) * 512]
                        if out_sb is kT:
                            nc.vector.tensor_scalar_add(
                                osl, psum, bias_sb[:, n:n + 1])
                        else:
                            nc.scalar.activation(
                                osl, psum, AF.Identity,
                                bias=bias_sb[:, n:n + 1])

        mtiles = {}
        if is_causal:
            dmt = sing.tile([P, P], bf16, tag="dmt")
            nc.gpsimd.dma_start(dmt, maskm_d[0:P, 0:P])
            for i in range(NT):
                mtiles[(i, i)] = dmt
        else:
            for i in range(NT):
                for j in range(NT):
                    if status[i][j] == "part":
                        t = sing.tile([P, P], bf16, tag=f"mt{i}_{j}",
                                      name=f"mt{i}_{j}")
                        nc.sync.dma_start(
                            t, maskm_d[i * P:(i + 1) * P, j * P:(j + 1) * P]
                        )
                        mtiles[(i, j)] = t

        # ---- V projection -> V_plus [128, 16, 65] per token tile ---------
        vps = []
        for i in range(NT):
            vp = sing.tile([P, NH, DEP + 1], bf16, tag=f"vp{i}", name=f"vp{i}")
            nc.vector.memset(vp[:, :, DEP:DEP + 1], 1.0)
            vps.append(vp)
        for dh in range(2):
            wt = wpool.tile([P, NJ, 512], bf16, tag="wbig")
            nc.sync.dma_start(wt, wv_v[:, :, dh * 512:(dh + 1) * 512])
            for i in range(NT):
                psum = ps_mm.tile([P, 512], f32, tag="mm")
                for j in range(NJ):
                    nc.tensor.matmul(
                        psum,
                        xTb[:, j, i * P:(i + 1) * P],
                        wt[:, j, :],
                        start=(j == 0), stop=(j == NJ - 1),
                    )
                nc.vector.tensor_add(
                    out=vps[i][:, dh * 8:(dh + 1) * 8, 0:DEP],
                    in0=psum[:, :].rearrange("p (l d) -> p l d", d=DEP),
                    in1=bv_b[:, dh * 512:(dh + 1) * 512].rearrange(
                        "p (l d) -> p l d", d=DEP),
                )

        # Wo resident for the post-attention dense accumulation
        wo_sb = sing.tile([P, NJ, D], bf16, tag="wo_sb")
        nc.gpsimd.dma_start(wo_sb, wo_v[:, :, :])

        # free the projection PSUM banks so attention can double-buffer
        proj_ctx.close()

        # ---- attention ----------------------------------------------------
        # unnormalized ctx, feature-major [dep+sub | pair | tok], bf16
        ctxT = sing.tile([P, NJ, S], bf16, tag="ctxT")

        attn_ctx = ExitStack()
        ps_lg = attn_ctx.enter_context(
            tc.tile_pool(name="ps_lg", bufs=2, space="PSUM"))
        ps_ctx = attn_ctx.enter_context(
            tc.tile_pool(name="ps_ctx", bufs=2, space="PSUM"))
        for pair in range(NJ):
            for Hh in range(2):
                if is_causal:
                    i_list = [i for i in range(NT)
                              if any(status[i][j] != "skip"
                                     for j in range(Hh * 4, Hh * 4 + 4))]
                else:
                    i_list = list(range(NT))
                psc = ps_ctx.tile([P, 2, 512], f32, tag="ctx", name="psc")
                for idx, i in enumerate(i_list):
                    s0 = max(0, i * P - Hh * 512) if is_causal else 0
                    W = 512 - s0
                    # the two heads of the pair sit in array row-groups
                    # 0-63 / 64-127 -> adjacent K=64 matmuls run concurrently;
                    # both land in one 2-bank PSUM tile for a single exp
                    plg = ps_lg.tile([P, 2, 512], f32, tag="lg")
                    for sub in range(2):
                        pb = sub * DEP
                        nc.tensor.matmul(
                            plg[:, sub, s0:512],
                            kT[pb:pb + DEP, pair, i * P:(i + 1) * P],
                            qT[pb:pb + DEP, pair,
                               Hh * 512 + s0:(Hh + 1) * 512],
                            start=True, stop=True,
                        )
                    et = epool.tile([P, 2, 512], bf16, tag="e")
                    nc.scalar.activation(
                        et[:, :, s0:512], plg[:, :, s0:512], AF.Exp,
                        scale=0.125,
                    )
                    for j in range((Hh * 512 + s0) // P, Hh * 4 + 4):
                        if status[i][j] == "full":
                            continue
                        c = j * P - Hh * 512
                        mt = mtiles.get((i, j))
                        if mt is None:  # 'skip' inside computed range
                            nc.vector.memset(et[:, :, c:c + P], 0.0)
                        else:
                            nc.vector.tensor_mul(
                                et[:, :, c:c + P], et[:, :, c:c + P],
                                _rep2(mt),
                            )
                    for sub in range(2):
                        h = pair * 2 + sub
                        nc.tensor.matmul(
                            psc[0:DEP + 1, sub, s0:512],
                            vps[i][:, h, :],
                            et[:, sub, s0:512],
                            start=(idx == 0), stop=(idx == len(i_list) - 1),
                        )
                # evict unnormalized ctx (cross-partition bf16 copies) and
                # bounce the raw den rows to DRAM (PSUM is DMA-unreachable,
                # so they hop via a tiny SBUF tile)
                g = pair * 2 + Hh
                cols = slice(Hh * 512, (Hh + 1) * 512)
                nc.vector.tensor_copy(ctxT[0:DEP, pair, cols],
                                      psc[0:DEP, 0, :])
                nc.vector.tensor_copy(ctxT[DEP:P, pair, cols],
                                      psc[0:DEP, 1, :])
                dent = bcpool.tile([1, 2, 512], bf16, tag="dent", bufs=2)
                nc.vector.tensor_copy(dent, psc[DEP:DEP + 1, :, :])
                nc.gpsimd.dma_start(den_d[g, :, :], dent)

                if g % 8 == 7:
                    # batched reciprocal for groups g-7..g: one DVE recip
                    # over 16 partition rows, then per-group partition
                    # broadcast via DMA and in-place ctx normalize. Batch 0
                    # overlaps the second half of attention; batch 1's tail
                    # hides under Wo's pair-ordered accumulation.
                    g0 = g - 7
                    denb = rot.tile([16, 512], bf16, tag="sub")
                    nc.sync.dma_start(denb, den_d[g0:g0 + 8, :, :])
                    recb = rot.tile([16, 512], bf16, tag="sub")
                    _dve_recip(nc, recb, denb)
                    nc.sync.dma_start(rec_d[g0:g0 + 8, :, :], recb)
                    for gg in range(g0, g0 + 8):
                        pr, hh = gg // 2, gg % 2
                        ccols = slice(hh * 512, (hh + 1) * 512)
                        bcs = bcpool.tile([P, 512], bf16, tag="bc", bufs=2)
                        nc.sync.dma_start(
                            bcs, bcast2_ap(rec_d[gg, :, :], DEP))
                        nc.vector.tensor_mul(
                            ctxT[:, pr, ccols], ctxT[:, pr, ccols], bcs)

        attn_ctx.close()
        ps_mm = ctx.enter_context(
            tc.tile_pool(name="ps_mm2", bufs=2, space="PSUM"))

        # ---- Wo + bias + residual -> o1f (bf16), then LN1 -----------------
        o1f = bigp.tile([P, NJ, S], bf16, tag="o1f")
        for n in range(NJ):
            for Hh in range(2):
                cols = slice(Hh * 512, (Hh + 1) * 512)
                pw = ps_mm.tile([P, 512], f32, tag="mm")
                for pair in range(NJ):
                    nc.tensor.matmul(
                        pw,
                        wo_sb[:, pair, n * P:(n + 1) * P],
                        ctxT[:, pair, cols],
                        start=(pair == 0), stop=(pair == NJ - 1),
                    )
                nc.vector.scalar_tensor_tensor(
                    out=o1f[:, n, cols], in0=pw, scalar=boT[:, n:n + 1],
                    in1=xTb[:, n, cols], op0=ALU.add, op1=ALU.add,
                )

        # ---- LN1 (transposed; sums read bf16 o1f directly) ----------------
        ln_ctx = ExitStack()
        ps_ln = ln_ctx.enter_context(
            tc.tile_pool(name="ps_ln", bufs=1, space="PSUM"))
        ps_s = [ps_ln.tile([P, 512], f32, tag=f"lns{Hh}", name=f"lns{Hh}")
                for Hh in range(2)]
        ps_q = [ps_ln.tile([P, 512], f32, tag=f"lnq{Hh}", name=f"lnq{Hh}")
                for Hh in range(2)]
        o1n = sing.tile([P, NJ, S], bf16, tag="ko")
        for Hh in range(2):
            cols = slice(Hh * 512, (Hh + 1) * 512)
            sq = rot.tile([P, NJ, 512], bf16, tag="sq", bufs=1)
            nc.scalar.activation(sq, o1f[:, :, cols], AF.Square)
            for n in range(NJ):
                nc.tensor.matmul(ps_s[Hh], ones_bf, o1f[:, n, cols],
                                 start=(n == 0), stop=(n == NJ - 1))
                nc.tensor.matmul(ps_q[Hh], ones_bf, sq[:, n, :],
                                 start=(n == 0), stop=(n == NJ - 1))
            mean = statp.tile([P, 512], f32, tag="mean")
            nc.vector.tensor_scalar_mul(mean, ps_s[Hh], 1.0 / D)
            m2 = rot.tile([P, 512], f32, tag="sub")
            nc.vector.tensor_mul(m2, mean, mean)
            var = statp.tile([P, 512], f32, tag="var")
            nc.vector.scalar_tensor_tensor(
                out=var, in0=ps_q[Hh], scalar=1.0 / D, in1=m2,
                op0=ALU.mult, op1=ALU.subtract,
            )
            nc.scalar.activation(var, var, AF.Sqrt, bias=eps_t)
            nc.vector.reciprocal(var, var)  # rstd
            for j in range(NJ):
                sl = o1f[:, j, cols]
                sub = rot.tile([P, 512], f32, tag="sub")
                nc.vector.tensor_sub(sub, sl, mean)
                nc.vector.tensor_mul(sub, sub, var)
                nc.vector.tensor_scalar(
                    out=o1n[:, j, cols], in0=sub,
                    scalar1=g1T[:, j:j + 1], scalar2=be1T[:, j:j + 1],
                    op0=ALU.mult, op1=ALU.add,
                )

        # ---- FFN + LN3 (FFN2 swapped -> natural layout) -------------------
        ln_ctx.close()

        b2_b = sing.tile([P, D], f32, tag="natb")
        nc.gpsimd.dma_start(b2_b, bcast_ap(b2_d[:], P))
        g3_b = sing.tile([P, D], f32, tag="g3_b")
        nc.gpsimd.dma_start(g3_b, bcast_ap(g3_d[:], P))
        be3_b = sing.tile([P, D], f32, tag="be3_b")
        nc.gpsimd.dma_start(be3_b, bcast_ap(be3_d[:], P))
        nat_ctx = ExitStack()
        ps_nat = nat_ctx.enter_context(
            tc.tile_pool(name="ps_nat", bufs=2, space="PSUM"))
        w2h = bigp.tile([P, W2RES, D], bf16, tag="bigx")
        nc.sync.dma_start(w2h, w2_v[:, :W2RES, :])
        for Hh in range(2):
            h1 = bigp.tile([P, NM, 512], bf16, tag="o1h")
            for nb in range(8):
                wt = wpool.tile([P, NJ, 512], bf16, tag="wbig")
                nc.sync.dma_start(wt, w1_v[:, :, nb * 512:(nb + 1) * 512])
                for mloc in range(4):
                    m = nb * 4 + mloc
                    psum = ps_mm.tile([P, 512], f32, tag="mm")
                    for j in range(NJ):
                        nc.tensor.matmul(
                            psum,
                            wt[:, j, mloc * P:(mloc + 1) * P],
                            o1n[:, j, Hh * 512:(Hh + 1) * 512],
                            start=(j == 0), stop=(j == NJ - 1),
                        )
                    nc.scalar.activation(
                        h1[:, m, :], psum, AF.Relu, bias=b1T[:, m:m + 1]
                    )
            for tp in range(2):
                pnats = [ps_nat.tile([P, D], f32, tag="nat", name=f"nat{Hh}{tp}{ti}")
                         for ti in range(2)]
                for m in range(NM):
                    if m < W2RES:
                        w2t = w2h[:, m, :]
                    else:
                        w2t = w2pool.tile([P, D], bf16, tag="w2t")
                        nc.gpsimd.dma_start(w2t, w2_v[:, m, :])
                    for ti in range(2):
                        tloc = tp * 2 + ti
                        for half in range(2):
                            nc.tensor.matmul(
                                pnats[ti][:, half * 512:(half + 1) * 512],
                                h1[:, m, tloc * P:(tloc + 1) * P],
                                w2t[:, half * 512:(half + 1) * 512],
                                start=(m == 0), stop=(m == NM - 1),
                            )
                for ti in range(2):
                    t = Hh * 4 + tp * 2 + ti
                    pnat = pnats[ti]
                    onat = onatp.tile([P, D], f32, tag="onat")
                    nc.scalar.activation(onat, pnat, AF.Copy)  # frees psum
                    nc.vector.tensor_add(onat, onat, b2_b)
                    stats = statp.tile([P, 2, 6], f32, tag="bnst")
                    nc.vector.bn_stats(stats[:, 0, :], onat[:, 0:512])
                    nc.vector.bn_stats(stats[:, 1, :], onat[:, 512:1024])
                    mv = statp.tile([P, 2], f32, tag="bnmv")
                    nc.vector.bn_aggr(mv, stats)
                    rs = statp.tile([P, 1], f32, tag="bnrs")
                    nc.scalar.activation(rs, mv[:, 1:2], AF.Sqrt, bias=eps_t)
                    nc.vector.reciprocal(rs, rs)
                    nc.vector.tensor_scalar(
                        out=onat, in0=onat, scalar1=mv[:, 0:1], scalar2=rs,
                        op0=ALU.subtract, op1=ALU.mult,
                    )
                    nc.vector.tensor_mul(onat, onat, g3_b)
                    obf = rot.tile([P, D], bf16, tag="sub")
                    nc.vector.tensor_add(obf, onat, be3_b)
                    nc.sync.dma_start(out_d[t * P:(t + 1) * P, :], obf)
        nat_ctx.close()

    return nc


_BUILD_CACHE = {}


def _get_nc(mask_qk):
    key = mask_qk.tobytes()
    if key not in _BUILD_CACHE:
        status, is_causal = _block_plan(mask_qk)
        nc = build_nc(status, is_causal)
        legalize_waits(nc)
        _BUILD_CACHE[key] = nc
    return _BUILD_CACHE[key]


def kernel(x, look_ahead_mask, wq, bq, wk, bk, wv, bv, wo, bo,
           w1, b1, w2, b2, ln1_g, ln1_b, ln3_g, ln3_b):
    global LAST_EXEC_NS
    x = np.asarray(x, dtype=np.float32)
    B = x.shape[0]
    mask = np.asarray(look_ahead_mask, dtype=np.float32)[0, 0]
    mask_qk = mask == 0.0  # True where key visible to query
    maskm = np.ascontiguousarray(mask_qk.T).astype(ml_dtypes.bfloat16)

    nc = _get_nc(mask_qk)

    bf = ml_dtypes.bfloat16
    shared = {
        "wq": np.ascontiguousarray(wq).astype(bf),
        "wk": np.ascontiguousarray(wk).astype(bf),
        "wv": np.ascontiguousarray(wv).astype(bf),
        "wo": np.ascontiguousarray(wo).astype(bf),
        "w1": np.ascontiguousarray(w1).astype(bf),
        "w2": np.ascontiguousarray(w2).astype(bf),
        "bq": np.asarray(bq, np.float32), "bk": np.asarray(bk, np.float32),
        "bv": np.asarray(bv, np.float32), "bo": np.asarray(bo, np.float32),
        "b1": np.asarray(b1, np.float32), "b2": np.asarray(b2, np.float32),
        "g1": np.asarray(ln1_g, np.float32),
        "be1": np.asarray(ln1_b, np.float32),
        "g3": np.asarray(ln3_g, np.float32),
        "be3": np.asarray(ln3_b, np.float32),
        "maskm": maskm,
    }
    in_maps = []
    for c in range(8):
        m = dict(shared)
        m["xTb"] = np.ascontiguousarray(x[c % B].T).astype(bf)
        in_maps.append(m)

    kwargs = {}
    if TRACE:
        kwargs = {"trace": True, "tmpdir": TRACE_DIR}
    res = run_bass_kernel_spmd(nc, in_maps, list(range(8)), **kwargs)
    LAST_EXEC_NS = res.exec_time_ns
    out = np.stack([res.results[c]["out"] for c in range(B)], axis=0)
    return out.astype(np.float32)


# revision 35
# speedup vs baseline: 1.1571x; 1.0423x over previous
"""Trainium2 Bass kernel for nn_DecoderLayer (B=8, S=1024, D=1024, H=16, DFF=4096).

Sharding: pure data-parallel over batch — one batch element per NeuronCore,
no collectives. Each core computes the full decoder layer for its element.

Per-core dataflow (activations kept feature-major, i.e. transposed [feat, tok]):
  qT/kT = W^T @ xT          (head-pair-major [128, 8, 1024] bf16)
  V_plus = xT^T @ Wv | ones (natural [tok, head, 64+1])
  per (pair, tok-half): logits for both heads of the pair land in one
  [128, 2, 512] PSUM tile (concurrent K=64 row-group matmuls), ONE exp
  per i-tile covers both heads; causal blocks skipped, partial blocks
  masked by 0/1 multiplier tiles.  [ctx; den] = V_plus^T @ E^T comes out
  unnormalized; ctx is evicted to SBUF bf16, den rows bounce through DRAM.
  After the last group: ONE batched ACT reciprocal over all 32 den rows
  (single table swap for the whole kernel), DMA partition-broadcast of
  the reciprocals, bf16 DVE muls normalize ctx in place.
  Wo then runs as a dense K=1024 PSUM accumulation (+bias+residual in one
  scalar_tensor_tensor), o1f kept bf16 so LN1's token sums read it
  directly; squares batched into one 3-D ACT op per token half.
  FFN1 transposed; FFN2 with swapped matmul operands -> natural [tok, feat];
  LN3 natively on PSUM (bn_stats), DMA out in natural layout.

SBUF slots are chained across phases via shared pool tags:
  xTb -> w2 resident;  qT -> h1;  kT -> o1n.
"""
import sys

sys.path.insert(0, "/opt/trn_rl_repo")

from contextlib import ExitStack

import numpy as np
import ml_dtypes

import concourse.bass as bass
import concourse.tile as tile
from concourse import mybir
from concourse.bass_utils import run_bass_kernel_spmd
from concourse.vector_clock import ScopedClock

P = 128
D = 1024
NH = 16
DEP = 64
DFF = 4096
S = 1024
NJ = D // P      # 8 feature tiles
NT = S // P      # 8 token tiles
NM = DFF // P    # 32 dff tiles
NG = NJ * 2      # 16 (pair, token-half) attention groups
EPS = 1e-6
W2RES = 8        # m-tiles of W2 kept resident in SBUF

f32 = mybir.dt.float32
bf16 = mybir.dt.bfloat16
AF = mybir.ActivationFunctionType
ALU = mybir.AluOpType

# host-side knobs (test.py may set TRACE=True for a profiled run)
TRACE = False
TRACE_DIR = None
LAST_EXEC_NS = None


class TileCtx(tile.TileContext):
    """This container's walrus rejects any instruction carrying >1 sync-wait.
    Split the kernel-tail drain's global-clock waits across single-wait
    Drains."""

    def _drain_and_barrier(self, tick_clock, wait_clock):
        nc = self.nc
        drain_inst = nc.sync.drain()
        wait_clock.add_sem_waits(
            drain_inst.ins, ScopedClock({None: tick_clock.global_clock})
        )
        waits = list(drain_inst.ins.sync_info.on_wait or [])
        if len(waits) > 1:
            del drain_inst.ins.sync_info.on_wait[1:]
            for w in waits[1:]:
                d = nc.sync.drain()
                if d.ins.sync_info is None:
                    d.ins.sync_info = mybir.SyncInfo(on_wait=[], on_update=[])
                d.ins.sync_info.on_wait.append(w)

        nc.all_engine_barrier()
        assert self.sems is not None
        popped = nc._tile_sem_poison_stack.pop()
        assert popped is self._sem_poison
        nc.clear_and_free_semaphores(list(self.sems.allocated().values()))
        nc.all_engine_barrier()


def legalize_waits(nc):
    """Split every multi-wait instruction into preceding single-wait Drains
    (same walrus limitation, applied to the whole program)."""
    import orjson

    bir = nc.to_json()
    ctr = 0
    for fn in bir["functions"]:
        for blk in fn["blocks"]:
            new = []
            for inst in blk["instructions"]:
                si = inst.get("sync_info")
                waits = (si or {}).get("on_wait") or []
                if len(waits) > 1:
                    for w in waits[:-1]:
                        ctr += 1
                        new.append({
                            "engine": inst["engine"],
                            "ins": [], "outs": [],
                            "name": f"I-wfix{ctr}",
                            "opcode": "NoOp",
                            "sync_info": {"on_update": [], "on_wait": [w]},
                            "debug": inst.get("debug"),
                        })
                    si["on_wait"] = [waits[-1]]
                new.append(inst)
            blk["instructions"] = new
    blob = orjson.dumps(bir)
    nc.to_json_bytes = lambda: blob
    return ctr


def _dve_recip(nc, out, in_):
    """DVE reciprocal with a low-precision (bf16) output; bass's wrapper
    fatals on bf16 out, but softmax denominators only need ~bf16 accuracy."""
    eng = nc.vector
    return eng.add_instruction(
        mybir.InstReciprocal(
            name=nc.get_next_instruction_name(),
            ins=[eng.lower_ap(in_)],
            outs=[eng.lower_ap(out)],
        )
    )


def _rep2(ap2d):
    """[128, N] AP -> [128, 2, N] AP with a stride-0 middle dim (same data
    fed to both heads of a pair)."""
    return bass.AP(
        tensor=ap2d.tensor, offset=ap2d.offset,
        ap=[list(ap2d.ap[0])] + [[0, 2]] + [list(p) for p in ap2d.ap[1:]],
    )


def _block_plan(mask_qk):
    """Classify [128 x 128] blocks of the visibility pattern.

    mask_qk: (S, S) bool, True where key k is VISIBLE to query q ([q, k]).
    Returns status[i][j] for sk-tile i, sq-tile j ('full'|'part'|'skip')
    and is_causal (enables narrow per-row column ranges).
    """
    vis_T = mask_qk.T  # [k, q]
    status = [[None] * NT for _ in range(NT)]
    for i in range(NT):
        for j in range(NT):
            blk = vis_T[i * P:(i + 1) * P, j * P:(j + 1) * P]
            status[i][j] = "full" if blk.all() else ("skip" if not blk.any()
                                                    else "part")
    causal = np.tril(np.ones((S, S), dtype=bool))
    return status, bool((mask_qk == causal).all())


def build_nc(status, is_causal):
    nc = bass.Bass()

    # ---- I/O -------------------------------------------------------------
    xTb_d = nc.declare_dram_parameter("xTb", [D, S], bf16, isOutput=False)
    wq_d = nc.declare_dram_parameter("wq", [D, D], bf16, isOutput=False)
    wk_d = nc.declare_dram_parameter("wk", [D, D], bf16, isOutput=False)
    wv_d = nc.declare_dram_parameter("wv", [D, D], bf16, isOutput=False)
    wo_d = nc.declare_dram_parameter("wo", [D, D], bf16, isOutput=False)
    w1_d = nc.declare_dram_parameter("w1", [D, DFF], bf16, isOutput=False)
    w2_d = nc.declare_dram_parameter("w2", [DFF, D], bf16, isOutput=False)
    bq_d = nc.declare_dram_parameter("bq", [D], f32, isOutput=False)
    bk_d = nc.declare_dram_parameter("bk", [D], f32, isOutput=False)
    bv_d = nc.declare_dram_parameter("bv", [D], f32, isOutput=False)
    bo_d = nc.declare_dram_parameter("bo", [D], f32, isOutput=False)
    b1_d = nc.declare_dram_parameter("b1", [DFF], f32, isOutput=False)
    b2_d = nc.declare_dram_parameter("b2", [D], f32, isOutput=False)
    g1_d = nc.declare_dram_parameter("g1", [D], f32, isOutput=False)
    be1_d = nc.declare_dram_parameter("be1", [D], f32, isOutput=False)
    g3_d = nc.declare_dram_parameter("g3", [D], f32, isOutput=False)
    be3_d = nc.declare_dram_parameter("be3", [D], f32, isOutput=False)
    maskm_d = nc.declare_dram_parameter("maskm", [S, S], bf16, isOutput=False)
    out_d = nc.declare_dram_parameter("out", [S, D], bf16, isOutput=True)

    # DRAM scratch for the softmax-denominator partition broadcast: raw den
    # rows bounce out per group, reciprocals bounce back per 8-group batch
    den_d = nc.dram_tensor("den_sc", [NG, 2, 512], bf16, kind="Internal")
    rec_d = nc.dram_tensor("rec_sc", [NG, 2, 512], bf16, kind="Internal")

    xTb_v = xTb_d[:, :].rearrange("(ko ki) t -> ki ko t", ki=P)
    wq_v = wq_d[:, :].rearrange("(ko ki) n -> ki ko n", ki=P)
    wk_v = wk_d[:, :].rearrange("(ko ki) n -> ki ko n", ki=P)
    wv_v = wv_d[:, :].rearrange("(ko ki) n -> ki ko n", ki=P)
    wo_v = wo_d[:, :].rearrange("(ko ki) n -> ki ko n", ki=P)
    w1_v = w1_d[:, :].rearrange("(ko ki) n -> ki ko n", ki=P)
    w2_v = w2_d[:, :].rearrange("(mo ki) n -> ki mo n", ki=P)

    def bcast_ap(src_1d, parts):
        """1-D DRAM AP [N] -> stride-0 partition-broadcast AP [parts, N]."""
        return bass.AP(
            tensor=src_1d.tensor, offset=src_1d.offset,
            ap=[[0, parts]] + [list(p) for p in src_1d.ap],
        )

    def bcast2_ap(src_2d, reps):
        """2-D DRAM AP [2, N] -> [2, reps, N] AP (each row replicated)."""
        return bass.AP(
            tensor=src_2d.tensor, offset=src_2d.offset,
            ap=[list(src_2d.ap[0])] + [[0, reps]] + [list(src_2d.ap[1])],
        )

    with TileCtx(nc) as tc, ExitStack() as ctx:
        sing = ctx.enter_context(tc.tile_pool(name="sing", bufs=1))
        bigp = ctx.enter_context(tc.tile_pool(name="bigp", bufs=1))
        wpool = ctx.enter_context(tc.tile_pool(name="wpool", bufs=3))
        rot = ctx.enter_context(tc.tile_pool(name="rot", bufs=2))
        statp = ctx.enter_context(tc.tile_pool(name="statp", bufs=2))
        epool = ctx.enter_context(tc.tile_pool(name="epool", bufs=3))
        bcpool = ctx.enter_context(tc.tile_pool(name="bcpool", bufs=3))
        onatp = ctx.enter_context(tc.tile_pool(name="onatp", bufs=2))
        proj_ctx = ExitStack()
        ps_mm = proj_ctx.enter_context(
            tc.tile_pool(name="ps_mm", bufs=2, space="PSUM"))

        # ---- constants / params ------------------------------------------
        # slot chain "bigx": xTb (2MB) -> w2 resident half (2MB)
        xTb = bigp.tile([P, NJ, S], bf16, tag="bigx")
        for j in range(NJ):
            eng = nc.sync if j % 2 == 0 else nc.gpsimd
            eng.dma_start(xTb[:, j, :], xTb_v[:, j, :])

        def load_bias_T(d_ap, ko, tag):
            t = sing.tile([P, ko], f32, tag=tag)
            nc.gpsimd.dma_start(t, d_ap[:].rearrange("(ko ki) -> ki ko", ki=P))
            return t

        bqT = load_bias_T(bq_d, NJ, "bqT")
        bkT = load_bias_T(bk_d, NJ, "bkT")
        boT = load_bias_T(bo_d, NJ, "boT")
        b1T = load_bias_T(b1_d, NM, "b1T")
        g1T = load_bias_T(g1_d, NJ, "g1T")
        be1T = load_bias_T(be1_d, NJ, "be1T")

        bv_b = sing.tile([P, D], f32, tag="natb")
        nc.gpsimd.dma_start(bv_b, bcast_ap(bv_d[:], P))
        ones_bf = sing.tile([P, P], bf16, tag="ones_bf")
        nc.vector.memset(ones_bf, 1.0)
        eps_t = sing.tile([P, 1], f32, tag="eps_t")
        nc.vector.memset(eps_t, EPS)

        # ---- Q/K projections: [128, 8(pair), 1024] bf16 ------------------
        # slot chain "o1h": qT (2MB) -> h1 x2 (4MB)
        qT = bigp.tile([P, NJ, S], bf16, tag="o1h")
        kT = sing.tile([P, NJ, S], bf16, tag="ko")
        for w_v, out_sb, bias_sb in ((wq_v, qT, bqT), (wk_v, kT, bkT)):
            for nb in range(2):
                wt = wpool.tile([P, NJ, 512], bf16, tag="wbig")
                for j in range(NJ):
                    eng = nc.sync if j % 2 == 0 else nc.gpsimd
                    eng.dma_start(
                        wt[:, j, :], w_v[:, j, nb * 512:(nb + 1) * 512])
                for nn in range(4):
                    n = nb * 4 + nn
                    for Hh in range(2):
                        psum = ps_mm.tile([P, 512], f32, tag="mm")
                        for j in range(NJ):
                            nc.tensor.matmul(
                                psum,
                                wt[:, j, nn * P:(nn + 1) * P],
                                xTb[:, j, Hh * 512:(Hh + 1) * 512],
                                start=(j == 0), stop=(j == NJ - 1),
                            )
                        osl = out_sb[:, n, Hh * 512:(Hh + 1) * 512]
                        if out_sb is kT:
                            nc.vector.tensor_scalar_add(
                                osl, psum, bias_sb[:, n:n + 1])
                        else:
                            nc.scalar.activation(
                                osl, psum, AF.Identity,
                                bias=bias_sb[:, n:n + 1])

        mtiles = {}
        if is_causal:
            dmt = sing.tile([P, P], bf16, tag="dmt")
            nc.gpsimd.dma_start(dmt, maskm_d[0:P, 0:P])
            for i in range(NT):
                mtiles[(i, i)] = dmt
        else:
            for i in range(NT):
                for j in range(NT):
                    if status[i][j] == "part":
                        t = sing.tile([P, P], bf16, tag=f"mt{i}_{j}",
                                      name=f"mt{i}_{j}")
                        nc.sync.dma_start(
                            t, maskm_d[i * P:(i + 1) * P, j * P:(j + 1) * P]
                        )
                        mtiles[(i, j)] = t

        # ---- V projection -> V_plus [128, 16, 65] per token tile ---------
        vps = []
        for i in range(NT):
            vp = sing.tile([P, NH, DEP + 1], bf16, tag=f"vp{i}", name=f"vp{i}")
            nc.vector.memset(vp[:, :, DEP:DEP + 1], 1.0)
            vps.append(vp)
        for dh in range(2):
            wt = wpool.tile([P, NJ, 512], bf16, tag="wbig")
            nc.sync.dma_start(wt, wv_v[:, :, dh * 512:(dh + 1) * 512])
            for i in range(NT):
                psum = ps_mm.tile([P, 512], f32, tag="mm")
                for j in range(NJ):
                    nc.tensor.matmul(
                        psum,
                        xTb[:, j, i * P:(i + 1) * P],
                        wt[:, j, :],
                        start=(j == 0), stop=(j == NJ - 1),
                    )
                nc.vector.tensor_add(
                    out=vps[i][:, dh * 8:(dh + 1) * 8, 0:DEP],
                    in0=psum[:, :].rearrange("p (l d) -> p l d", d=DEP),
                    in1=bv_b[:, dh * 512:(dh + 1) * 512].rearrange(
                        "p (l d) -> p l d", d=DEP),
                )

        # Wo resident for the post-attention dense accumulation
        wo_sb = sing.tile([P, NJ, D], bf16, tag="wo_sb")
        nc.gpsimd.dma_start(wo_sb, wo_v[:, :, :])

        # free the projection PSUM banks so attention can double-buffer
        proj_ctx.close()

        # ---- attention ----------------------------------------------------
        # unnormalized ctx, feature-major [dep+sub | pair | tok], bf16
        ctxT = sing.tile([P, NJ, S], bf16, tag="ctxT")

        attn_ctx = ExitStack()
        ps_lg = attn_ctx.enter_context(
            tc.tile_pool(name="ps_lg", bufs=2, space="PSUM"))
        ps_ctx = attn_ctx.enter_context(
            tc.tile_pool(name="ps_ctx", bufs=2, space="PSUM"))
        for pair in range(NJ):
            for Hh in range(2):
                if is_causal:
                    i_list = [i for i in range(NT)
                              if any(status[i][j] != "skip"
                                     for j in range(Hh * 4, Hh * 4 + 4))]
                else:
                    i_list = list(range(NT))
                psc = ps_ctx.tile([P, 2, 512], f32, tag="ctx", name="psc")
                for idx, i in enumerate(i_list):
                    s0 = max(0, i * P - Hh * 512) if is_causal else 0
                    W = 512 - s0
                    # the two heads of the pair sit in array row-groups
                    # 0-63 / 64-127 -> adjacent K=64 matmuls run concurrently;
                    # both land in one 2-bank PSUM tile for a single exp
                    plg = ps_lg.tile([P, 2, 512], f32, tag="lg")
                    for sub in range(2):
                        pb = sub * DEP
                        nc.tensor.matmul(
                            plg[:, sub, s0:512],
                            kT[pb:pb + DEP, pair, i * P:(i + 1) * P],
                            qT[pb:pb + DEP, pair,
                               Hh * 512 + s0:(Hh + 1) * 512],
                            start=True, stop=True,
                        )
                    et = epool.tile([P, 2, 512], bf16, tag="e")
                    nc.scalar.activation(
                        et[:, :, s0:512], plg[:, :, s0:512], AF.Exp,
                        scale=0.125,
                    )
                    for j in range((Hh * 512 + s0) // P, Hh * 4 + 4):
                        if status[i][j] == "full":
                            continue
                        c = j * P - Hh * 512
                        mt = mtiles.get((i, j))
                        if mt is None:  # 'skip' inside computed range
                            nc.vector.memset(et[:, :, c:c + P], 0.0)
                        else:
                            nc.vector.tensor_mul(
                                et[:, :, c:c + P], et[:, :, c:c + P],
                                _rep2(mt),
                            )
                    for sub in range(2):
                        h = pair * 2 + sub
                        nc.tensor.matmul(
                            psc[0:DEP + 1, sub, s0:512],
                            vps[i][:, h, :],
                            et[:, sub, s0:512],
                            start=(idx == 0), stop=(idx == len(i_list) - 1),
                        )
                # evict unnormalized ctx (cross-partition bf16 copies) and
                # bounce the raw den rows to DRAM (PSUM is DMA-unreachable,
                # so they hop via a tiny SBUF tile)
                g = pair * 2 + Hh
                cols = slice(Hh * 512, (Hh + 1) * 512)
                nc.vector.tensor_copy(ctxT[0:DEP, pair, cols],
                                      psc[0:DEP, 0, :])
                nc.vector.tensor_copy(ctxT[DEP:P, pair, cols],
                                      psc[0:DEP, 1, :])
                dent = bcpool.tile([1, 2, 512], bf16, tag="dent", bufs=2)
                nc.vector.tensor_copy(dent, psc[DEP:DEP + 1, :, :])
                nc.gpsimd.dma_start(den_d[g, :, :], dent)

                if g % 4 == 3:
                    # batched reciprocal for groups g-3..g: one DVE recip
                    # over 8 partition rows, then per-group partition
                    # broadcast via DMA and in-place ctx normalize. Early
                    # batches overlap the rest of attention; the last
                    # batch's tail hides under Wo's pair-ordered
                    # accumulation.
                    g0 = g - 3
                    denb = rot.tile([8, 512], bf16, tag="sub")
                    nc.sync.dma_start(denb, den_d[g0:g0 + 4, :, :])
                    recb = rot.tile([8, 512], bf16, tag="sub")
                    _dve_recip(nc, recb, denb)
                    nc.sync.dma_start(rec_d[g0:g0 + 4, :, :], recb)
                    for gg in range(g0, g0 + 4):
                        pr, hh = gg // 2, gg % 2
                        ccols = slice(hh * 512, (hh + 1) * 512)
                        bcs = bcpool.tile([P, 512], bf16, tag="bc", bufs=2)
                        nc.sync.dma_start(
                            bcs, bcast2_ap(rec_d[gg, :, :], DEP))
                        nc.vector.tensor_mul(
                            ctxT[:, pr, ccols], ctxT[:, pr, ccols], bcs)

        attn_ctx.close()
        ps_mm = ctx.enter_context(
            tc.tile_pool(name="ps_mm2", bufs=2, space="PSUM"))

        # ---- Wo + bias + residual -> o1f (bf16), then LN1 -----------------
        o1f = bigp.tile([P, NJ, S], bf16, tag="o1f")
        for n in range(NJ):
            for Hh in range(2):
                cols = slice(Hh * 512, (Hh + 1) * 512)
                pw = ps_mm.tile([P, 512], f32, tag="mm")
                for pair in range(NJ):
                    nc.tensor.matmul(
                        pw,
                        wo_sb[:, pair, n * P:(n + 1) * P],
                        ctxT[:, pair, cols],
                        start=(pair == 0), stop=(pair == NJ - 1),
                    )
                nc.vector.scalar_tensor_tensor(
                    out=o1f[:, n, cols], in0=pw, scalar=boT[:, n:n + 1],
                    in1=xTb[:, n, cols], op0=ALU.add, op1=ALU.add,
                )

        # ---- LN1 (transposed; sums read bf16 o1f directly) ----------------
        ln_ctx = ExitStack()
        ps_ln = ln_ctx.enter_context(
            tc.tile_pool(name="ps_ln", bufs=1, space="PSUM"))
        ps_s = [ps_ln.tile([P, 512], f32, tag=f"lns{Hh}", name=f"lns{Hh}")
                for Hh in range(2)]
        ps_q = [ps_ln.tile([P, 512], f32, tag=f"lnq{Hh}", name=f"lnq{Hh}")
                for Hh in range(2)]
        o1n = sing.tile([P, NJ, S], bf16, tag="ko")
        for Hh in range(2):
            cols = slice(Hh * 512, (Hh + 1) * 512)
            sq = rot.tile([P, NJ, 512], bf16, tag="sq", bufs=1)
            nc.scalar.activation(sq, o1f[:, :, cols], AF.Square)
            for n in range(NJ):
                nc.tensor.matmul(ps_s[Hh], ones_bf, o1f[:, n, cols],
                                 start=(n == 0), stop=(n == NJ - 1))
                nc.tensor.matmul(ps_q[Hh], ones_bf, sq[:, n, :],
                                 start=(n == 0), stop=(n == NJ - 1))
            mean = statp.tile([P, 512], f32, tag="mean")
            nc.vector.tensor_scalar_mul(mean, ps_s[Hh], 1.0 / D)
            m2 = rot.tile([P, 512], f32, tag="sub")
            nc.vector.tensor_mul(m2, mean, mean)
            var = statp.tile([P, 512], f32, tag="var")
            nc.vector.scalar_tensor_tensor(
                out=var, in0=ps_q[Hh], scalar=1.0 / D, in1=m2,
                op0=ALU.mult, op1=ALU.subtract,
            )
            nc.scalar.activation(var, var, AF.Sqrt, bias=eps_t)
            nc.vector.reciprocal(var, var)  # rstd
            for j in range(NJ):
                sl = o1f[:, j, cols]
                sub = rot.tile([P, 512], f32, tag="sub")
                nc.vector.tensor_sub(sub, sl, mean)
                nc.vector.tensor_mul(sub, sub, var)
                nc.vector.tensor_scalar(
                    out=o1n[:, j, cols], in0=sub,
                    scalar1=g1T[:, j:j + 1], scalar2=be1T[:, j:j + 1],
                    op0=ALU.mult, op1=ALU.add,
                )

        # ---- FFN + LN3 (FFN2 swapped -> natural layout) -------------------
        ln_ctx.close()

        b2_b = sing.tile([P, D], f32, tag="natb")
        nc.gpsimd.dma_start(b2_b, bcast_ap(b2_d[:], P))
        g3_b = sing.tile([P, D], f32, tag="g3_b")
        nc.gpsimd.dma_start(g3_b, bcast_ap(g3_d[:], P))
        be3_b = sing.tile([P, D], f32, tag="be3_b")
        nc.gpsimd.dma_start(be3_b, bcast_ap(be3_d[:], P))
        nat_ctx = ExitStack()
        ps_nat = nat_ctx.enter_context(
            tc.tile_pool(name="ps_nat", bufs=2, space="PSUM"))
        # W2 fully resident: after Wo the xTb / vps / ctxT / wo_sb slots are
        # dead — exactly 64KB/partition — so the whole of W2 moves into them
        # during LN1/FFN1 and FFN2 streams nothing.
        w2h = bigp.tile([P, 8, D], bf16, tag="bigx")
        nc.sync.dma_start(w2h, w2_v[:, 0:8, :])
        w2x = []
        for i in range(8):
            t = sing.tile([P, D], bf16, tag=f"vp{i}", name=f"w2x{i}")
            nc.gpsimd.dma_start(t, w2_v[:, 8 + i, :])
            w2x.append(t)
        w2c = sing.tile([P, 8, D], bf16, tag="ctxT")
        nc.sync.dma_start(w2c, w2_v[:, 16:24, :])
        w2w = sing.tile([P, 8, D], bf16, tag="wo_sb")
        nc.gpsimd.dma_start(w2w, w2_v[:, 24:32, :])

        def w2_res(m):
            if m < 8:
                return w2h[:, m, :]
            if m < 16:
                return w2x[m - 8]
            if m < 24:
                return w2c[:, m - 16, :]
            return w2w[:, m - 24, :]

        for Hh in range(2):
            h1 = bigp.tile([P, NM, 512], bf16, tag="o1h")
            for nb in range(8):
                wt = wpool.tile([P, NJ, 512], bf16, tag="wbig")
                nc.sync.dma_start(wt, w1_v[:, :, nb * 512:(nb + 1) * 512])
                for mloc in range(4):
                    m = nb * 4 + mloc
                    psum = ps_mm.tile([P, 512], f32, tag="mm")
                    for j in range(NJ):
                        nc.tensor.matmul(
                            psum,
                            wt[:, j, mloc * P:(mloc + 1) * P],
                            o1n[:, j, Hh * 512:(Hh + 1) * 512],
                            start=(j == 0), stop=(j == NJ - 1),
                        )
                    nc.scalar.activation(
                        h1[:, m, :], psum, AF.Relu, bias=b1T[:, m:m + 1]
                    )
            for tp in range(2):
                pnats = [ps_nat.tile([P, D], f32, tag="nat", name=f"nat{Hh}{tp}{ti}")
                         for ti in range(2)]
                for m in range(NM):
                    w2t = w2_res(m)
                    for ti in range(2):
                        tloc = tp * 2 + ti
                        for half in range(2):
                            nc.tensor.matmul(
                                pnats[ti][:, half * 512:(half + 1) * 512],
                                h1[:, m, tloc * P:(tloc + 1) * P],
                                w2t[:, half * 512:(half + 1) * 512],
                                start=(m == 0), stop=(m == NM - 1),
                            )
                for ti in range(2):
                    t = Hh * 4 + tp * 2 + ti
                    pnat = pnats[ti]
                    onat = onatp.tile([P, D], f32, tag="onat")
                    nc.scalar.activation(onat, pnat, AF.Copy)  # frees psum
                    nc.vector.tensor_add(onat, onat, b2_b)
                    stats = statp.tile([P, 2, 6], f32, tag="bnst")
                    nc.vector.bn_stats(stats[:, 0, :], onat[:, 0:512])
                    nc.vector.bn_stats(stats[:, 1, :], onat[:, 512:1024])
                    mv = statp.tile([P, 2], f32, tag="bnmv")
                    nc.vector.bn_aggr(mv, stats)
                    rs = statp.tile([P, 1], f32, tag="bnrs")
                    nc.scalar.activation(rs, mv[:, 1:2], AF.Sqrt, bias=eps_t)
                    nc.vector.reciprocal(rs, rs)
                    nc.vector.tensor_scalar(
                        out=onat, in0=onat, scalar1=mv[:, 0:1], scalar2=rs,
                        op0=ALU.subtract, op1=ALU.mult,
                    )
                    nc.vector.tensor_mul(onat, onat, g3_b)
                    obf = rot.tile([P, D], bf16, tag="sub")
                    nc.vector.tensor_add(obf, onat, be3_b)
                    nc.sync.dma_start(out_d[t * P:(t + 1) * P, :], obf)
        nat_ctx.close()

    return nc


_BUILD_CACHE = {}


def _get_nc(mask_qk):
    key = mask_qk.tobytes()
    if key not in _BUILD_CACHE:
        status, is_causal = _block_plan(mask_qk)
        nc = build_nc(status, is_causal)
        legalize_waits(nc)
        _BUILD_CACHE[key] = nc
    return _BUILD_CACHE[key]


def kernel(x, look_ahead_mask, wq, bq, wk, bk, wv, bv, wo, bo,
           w1, b1, w2, b2, ln1_g, ln1_b, ln3_g, ln3_b):
    global LAST_EXEC_NS
    x = np.asarray(x, dtype=np.float32)
    B = x.shape[0]
    mask = np.asarray(look_ahead_mask, dtype=np.float32)[0, 0]
    mask_qk = mask == 0.0  # True where key visible to query
    maskm = np.ascontiguousarray(mask_qk.T).astype(ml_dtypes.bfloat16)

    nc = _get_nc(mask_qk)

    bf = ml_dtypes.bfloat16
    shared = {
        "wq": np.ascontiguousarray(wq).astype(bf),
        "wk": np.ascontiguousarray(wk).astype(bf),
        "wv": np.ascontiguousarray(wv).astype(bf),
        "wo": np.ascontiguousarray(wo).astype(bf),
        "w1": np.ascontiguousarray(w1).astype(bf),
        "w2": np.ascontiguousarray(w2).astype(bf),
        "bq": np.asarray(bq, np.float32), "bk": np.asarray(bk, np.float32),
        "bv": np.asarray(bv, np.float32), "bo": np.asarray(bo, np.float32),
        "b1": np.asarray(b1, np.float32), "b2": np.asarray(b2, np.float32),
        "g1": np.asarray(ln1_g, np.float32),
        "be1": np.asarray(ln1_b, np.float32),
        "g3": np.asarray(ln3_g, np.float32),
        "be3": np.asarray(ln3_b, np.float32),
        "maskm": maskm,
    }
    in_maps = []
    for c in range(8):
        m = dict(shared)
        m["xTb"] = np.ascontiguousarray(x[c % B].T).astype(bf)
        in_maps.append(m)

    kwargs = {}
    if TRACE:
        kwargs = {"trace": True, "tmpdir": TRACE_DIR}
    res = run_bass_kernel_spmd(nc, in_maps, list(range(8)), **kwargs)
    LAST_EXEC_NS = res.exec_time_ns
    out = np.stack([res.results[c]["out"] for c in range(B)], axis=0)
    return out.astype(np.float32)


# revision 37
# speedup vs baseline: 1.1693x; 1.0105x over previous
"""Trainium2 Bass kernel for nn_DecoderLayer (B=8, S=1024, D=1024, H=16, DFF=4096).

Sharding: pure data-parallel over batch — one batch element per NeuronCore,
no collectives. Each core computes the full decoder layer for its element.

Per-core dataflow (activations kept feature-major, i.e. transposed [feat, tok]):
  qT/kT = W^T @ xT          (head-pair-major [128, 8, 1024] bf16)
  V_plus = xT^T @ Wv | ones (natural [tok, head, 64+1])
  per (pair, tok-half): logits for both heads of the pair land in one
  [128, 2, 512] PSUM tile (concurrent K=64 row-group matmuls), ONE exp
  per i-tile covers both heads; causal blocks skipped, partial blocks
  masked by 0/1 multiplier tiles.  [ctx; den] = V_plus^T @ E^T comes out
  unnormalized; ctx is evicted to SBUF bf16, den rows bounce through DRAM.
  After the last group: ONE batched ACT reciprocal over all 32 den rows
  (single table swap for the whole kernel), DMA partition-broadcast of
  the reciprocals, bf16 DVE muls normalize ctx in place.
  Wo then runs as a dense K=1024 PSUM accumulation (+bias+residual in one
  scalar_tensor_tensor), o1f kept bf16 so LN1's token sums read it
  directly; squares batched into one 3-D ACT op per token half.
  FFN1 transposed; FFN2 with swapped matmul operands -> natural [tok, feat];
  LN3 natively on PSUM (bn_stats), DMA out in natural layout.

SBUF slots are chained across phases via shared pool tags:
  xTb -> w2 resident;  qT -> h1;  kT -> o1n.
"""
import sys

sys.path.insert(0, "/opt/trn_rl_repo")

from contextlib import ExitStack

import numpy as np
import ml_dtypes

import concourse.bass as bass
import concourse.tile as tile
from concourse import mybir
from concourse.bass_utils import run_bass_kernel_spmd
from concourse.vector_clock import ScopedClock

P = 128
D = 1024
NH = 16
DEP = 64
DFF = 4096
S = 1024
NJ = D // P      # 8 feature tiles
NT = S // P      # 8 token tiles
NM = DFF // P    # 32 dff tiles
NG = NJ * 2      # 16 (pair, token-half) attention groups
EPS = 1e-6
W2RES = 8        # m-tiles of W2 kept resident in SBUF

f32 = mybir.dt.float32
bf16 = mybir.dt.bfloat16
AF = mybir.ActivationFunctionType
ALU = mybir.AluOpType

# host-side knobs (test.py may set TRACE=True for a profiled run)
TRACE = False
TRACE_DIR = None
LAST_EXEC_NS = None


class TileCtx(tile.TileContext):
    """This container's walrus rejects any instruction carrying >1 sync-wait.
    Split the kernel-tail drain's global-clock waits across single-wait
    Drains."""

    def _drain_and_barrier(self, tick_clock, wait_clock):
        nc = self.nc
        drain_inst = nc.sync.drain()
        wait_clock.add_sem_waits(
            drain_inst.ins, ScopedClock({None: tick_clock.global_clock})
        )
        waits = list(drain_inst.ins.sync_info.on_wait or [])
        if len(waits) > 1:
            del drain_inst.ins.sync_info.on_wait[1:]
            for w in waits[1:]:
                d = nc.sync.drain()
                if d.ins.sync_info is None:
                    d.ins.sync_info = mybir.SyncInfo(on_wait=[], on_update=[])
                d.ins.sync_info.on_wait.append(w)

        nc.all_engine_barrier()
        assert self.sems is not None
        popped = nc._tile_sem_poison_stack.pop()
        assert popped is self._sem_poison
        nc.clear_and_free_semaphores(list(self.sems.allocated().values()))
        nc.all_engine_barrier()


def legalize_waits(nc):
    """Split every multi-wait instruction into preceding single-wait Drains
    (same walrus limitation, applied to the whole program)."""
    import orjson

    bir = nc.to_json()
    ctr = 0
    for fn in bir["functions"]:
        for blk in fn["blocks"]:
            new = []
            for inst in blk["instructions"]:
                si = inst.get("sync_info")
                waits = (si or {}).get("on_wait") or []
                if len(waits) > 1:
                    for w in waits[:-1]:
                        ctr += 1
                        new.append({
                            "engine": inst["engine"],
                            "ins": [], "outs": [],
                            "name": f"I-wfix{ctr}",
                            "opcode": "NoOp",
                            "sync_info": {"on_update": [], "on_wait": [w]},
                            "debug": inst.get("debug"),
                        })
                    si["on_wait"] = [waits[-1]]
                new.append(inst)
            blk["instructions"] = new
    blob = orjson.dumps(bir)
    nc.to_json_bytes = lambda: blob
    return ctr


def _dve_recip(nc, out, in_):
    """DVE reciprocal with a low-precision (bf16) output; bass's wrapper
    fatals on bf16 out, but softmax denominators only need ~bf16 accuracy."""
    eng = nc.vector
    return eng.add_instruction(
        mybir.InstReciprocal(
            name=nc.get_next_instruction_name(),
            ins=[eng.lower_ap(in_)],
            outs=[eng.lower_ap(out)],
        )
    )


def _rep2(ap2d):
    """[128, N] AP -> [128, 2, N] AP with a stride-0 middle dim (same data
    fed to both heads of a pair)."""
    return bass.AP(
        tensor=ap2d.tensor, offset=ap2d.offset,
        ap=[list(ap2d.ap[0])] + [[0, 2]] + [list(p) for p in ap2d.ap[1:]],
    )


def _block_plan(mask_qk):
    """Classify [128 x 128] blocks of the visibility pattern.

    mask_qk: (S, S) bool, True where key k is VISIBLE to query q ([q, k]).
    Returns status[i][j] for sk-tile i, sq-tile j ('full'|'part'|'skip')
    and is_causal (enables narrow per-row column ranges).
    """
    vis_T = mask_qk.T  # [k, q]
    status = [[None] * NT for _ in range(NT)]
    for i in range(NT):
        for j in range(NT):
            blk = vis_T[i * P:(i + 1) * P, j * P:(j + 1) * P]
            status[i][j] = "full" if blk.all() else ("skip" if not blk.any()
                                                    else "part")
    causal = np.tril(np.ones((S, S), dtype=bool))
    return status, bool((mask_qk == causal).all())


def build_nc(status, is_causal):
    nc = bass.Bass()

    # ---- I/O -------------------------------------------------------------
    xTb_d = nc.declare_dram_parameter("xTb", [D, S], bf16, isOutput=False)
    wq_d = nc.declare_dram_parameter("wq", [D, D], bf16, isOutput=False)
    wk_d = nc.declare_dram_parameter("wk", [D, D], bf16, isOutput=False)
    wv_d = nc.declare_dram_parameter("wv", [D, D], bf16, isOutput=False)
    wo_d = nc.declare_dram_parameter("wo", [D, D], bf16, isOutput=False)
    w1_d = nc.declare_dram_parameter("w1", [D, DFF], bf16, isOutput=False)
    w2_d = nc.declare_dram_parameter("w2", [DFF, D], bf16, isOutput=False)
    bq_d = nc.declare_dram_parameter("bq", [D], f32, isOutput=False)
    bk_d = nc.declare_dram_parameter("bk", [D], f32, isOutput=False)
    bv_d = nc.declare_dram_parameter("bv", [D], f32, isOutput=False)
    bo_d = nc.declare_dram_parameter("bo", [D], f32, isOutput=False)
    b1_d = nc.declare_dram_parameter("b1", [DFF], f32, isOutput=False)
    b2_d = nc.declare_dram_parameter("b2", [D], f32, isOutput=False)
    g1_d = nc.declare_dram_parameter("g1", [D], f32, isOutput=False)
    be1_d = nc.declare_dram_parameter("be1", [D], f32, isOutput=False)
    g3_d = nc.declare_dram_parameter("g3", [D], f32, isOutput=False)
    be3_d = nc.declare_dram_parameter("be3", [D], f32, isOutput=False)
    maskm_d = nc.declare_dram_parameter("maskm", [S, S], bf16, isOutput=False)
    out_d = nc.declare_dram_parameter("out", [S, D], bf16, isOutput=True)

    # DRAM scratch for the softmax-denominator partition broadcast: raw den
    # rows bounce out per group, reciprocals bounce back per 8-group batch
    den_d = nc.dram_tensor("den_sc", [NG, 2, 512], bf16, kind="Internal")
    rec_d = nc.dram_tensor("rec_sc", [NG, 2, 512], bf16, kind="Internal")

    xTb_v = xTb_d[:, :].rearrange("(ko ki) t -> ki ko t", ki=P)
    wq_v = wq_d[:, :].rearrange("(ko ki) n -> ki ko n", ki=P)
    wk_v = wk_d[:, :].rearrange("(ko ki) n -> ki ko n", ki=P)
    wv_v = wv_d[:, :].rearrange("(ko ki) n -> ki ko n", ki=P)
    wo_v = wo_d[:, :].rearrange("(ko ki) n -> ki ko n", ki=P)
    w1_v = w1_d[:, :].rearrange("(ko ki) n -> ki ko n", ki=P)
    w2_v = w2_d[:, :].rearrange("(mo ki) n -> ki mo n", ki=P)

    def bcast_ap(src_1d, parts):
        """1-D DRAM AP [N] -> stride-0 partition-broadcast AP [parts, N]."""
        return bass.AP(
            tensor=src_1d.tensor, offset=src_1d.offset,
            ap=[[0, parts]] + [list(p) for p in src_1d.ap],
        )

    def bcast2_ap(src_2d, reps):
        """2-D DRAM AP [2, N] -> [2, reps, N] AP (each row replicated)."""
        return bass.AP(
            tensor=src_2d.tensor, offset=src_2d.offset,
            ap=[list(src_2d.ap[0])] + [[0, reps]] + [list(src_2d.ap[1])],
        )

    with TileCtx(nc) as tc, ExitStack() as ctx:
        sing = ctx.enter_context(tc.tile_pool(name="sing", bufs=1))
        bigp = ctx.enter_context(tc.tile_pool(name="bigp", bufs=1))
        wpool = ctx.enter_context(tc.tile_pool(name="wpool", bufs=3))
        rot = ctx.enter_context(tc.tile_pool(name="rot", bufs=2))
        statp = ctx.enter_context(tc.tile_pool(name="statp", bufs=2))
        epool = ctx.enter_context(tc.tile_pool(name="epool", bufs=4))
        bcpool = ctx.enter_context(tc.tile_pool(name="bcpool", bufs=3))
        onatp = ctx.enter_context(tc.tile_pool(name="onatp", bufs=2))
        proj_ctx = ExitStack()
        ps_mm = proj_ctx.enter_context(
            tc.tile_pool(name="ps_mm", bufs=2, space="PSUM"))

        # ---- constants / params ------------------------------------------
        # slot chain "bigx": xTb (2MB) -> w2 resident half (2MB)
        xTb = bigp.tile([P, NJ, S], bf16, tag="bigx")
        for j in range(NJ):
            eng = nc.sync if j % 2 == 0 else nc.gpsimd
            eng.dma_start(xTb[:, j, :], xTb_v[:, j, :])

        def load_bias_T(d_ap, ko, tag):
            t = sing.tile([P, ko], f32, tag=tag)
            nc.gpsimd.dma_start(t, d_ap[:].rearrange("(ko ki) -> ki ko", ki=P))
            return t

        bqT = load_bias_T(bq_d, NJ, "bqT")
        bkT = load_bias_T(bk_d, NJ, "bkT")
        boT = load_bias_T(bo_d, NJ, "boT")
        b1T = load_bias_T(b1_d, NM, "b1T")
        g1T = load_bias_T(g1_d, NJ, "g1T")
        be1T = load_bias_T(be1_d, NJ, "be1T")

        bv_b = sing.tile([P, D], f32, tag="natb")
        nc.gpsimd.dma_start(bv_b, bcast_ap(bv_d[:], P))
        ones_bf = sing.tile([P, P], bf16, tag="ones_bf")
        nc.vector.memset(ones_bf, 1.0)
        eps_t = sing.tile([P, 1], f32, tag="eps_t")
        nc.vector.memset(eps_t, EPS)

        # ---- Q/K projections: [128, 8(pair), 1024] bf16 ------------------
        # slot chain "o1h": qT (2MB) -> h1 x2 (4MB)
        qT = bigp.tile([P, NJ, S], bf16, tag="o1h")
        kT = sing.tile([P, NJ, S], bf16, tag="ko")
        for w_v, out_sb, bias_sb in ((wq_v, qT, bqT), (wk_v, kT, bkT)):
            for nb in range(2):
                wt = wpool.tile([P, NJ, 512], bf16, tag="wbig")
                for j in range(NJ):
                    eng = nc.sync if j % 2 == 0 else nc.gpsimd
                    eng.dma_start(
                        wt[:, j, :], w_v[:, j, nb * 512:(nb + 1) * 512])
                for nn in range(4):
                    n = nb * 4 + nn
                    for Hh in range(2):
                        psum = ps_mm.tile([P, 512], f32, tag="mm")
                        for j in range(NJ):
                            nc.tensor.matmul(
                                psum,
                                wt[:, j, nn * P:(nn + 1) * P],
                                xTb[:, j, Hh * 512:(Hh + 1) * 512],
                                start=(j == 0), stop=(j == NJ - 1),
                            )
                        osl = out_sb[:, n, Hh * 512:(Hh + 1) * 512]
                        if out_sb is kT:
                            nc.vector.tensor_scalar_add(
                                osl, psum, bias_sb[:, n:n + 1])
                        else:
                            nc.scalar.activation(
                                osl, psum, AF.Identity,
                                bias=bias_sb[:, n:n + 1])

        mtiles = {}
        if is_causal:
            dmt = sing.tile([P, P], bf16, tag="dmt")
            nc.gpsimd.dma_start(dmt, maskm_d[0:P, 0:P])
            for i in range(NT):
                mtiles[(i, i)] = dmt
        else:
            for i in range(NT):
                for j in range(NT):
                    if status[i][j] == "part":
                        t = sing.tile([P, P], bf16, tag=f"mt{i}_{j}",
                                      name=f"mt{i}_{j}")
                        nc.sync.dma_start(
                            t, maskm_d[i * P:(i + 1) * P, j * P:(j + 1) * P]
                        )
                        mtiles[(i, j)] = t

        # ---- V projection -> V_plus [128, 16, 65] per token tile ---------
        vps = []
        for i in range(NT):
            vp = sing.tile([P, NH, DEP + 1], bf16, tag=f"vp{i}", name=f"vp{i}")
            nc.vector.memset(vp[:, :, DEP:DEP + 1], 1.0)
            vps.append(vp)
        for dh in range(2):
            wt = wpool.tile([P, NJ, 512], bf16, tag="wbig")
            nc.sync.dma_start(wt, wv_v[:, :, dh * 512:(dh + 1) * 512])
            for i in range(NT):
                psum = ps_mm.tile([P, 512], f32, tag="mm")
                for j in range(NJ):
                    nc.tensor.matmul(
                        psum,
                        xTb[:, j, i * P:(i + 1) * P],
                        wt[:, j, :],
                        start=(j == 0), stop=(j == NJ - 1),
                    )
                nc.vector.tensor_add(
                    out=vps[i][:, dh * 8:(dh + 1) * 8, 0:DEP],
                    in0=psum[:, :].rearrange("p (l d) -> p l d", d=DEP),
                    in1=bv_b[:, dh * 512:(dh + 1) * 512].rearrange(
                        "p (l d) -> p l d", d=DEP),
                )

        # Wo resident for the post-attention dense accumulation
        wo_sb = sing.tile([P, NJ, D], bf16, tag="wo_sb")
        nc.gpsimd.dma_start(wo_sb, wo_v[:, :, :])

        # free the projection PSUM banks so attention can double-buffer
        proj_ctx.close()

        # ---- attention ----------------------------------------------------
        # unnormalized ctx, feature-major [dep+sub | pair | tok], bf16
        ctxT = sing.tile([P, NJ, S], bf16, tag="ctxT")

        attn_ctx = ExitStack()
        ps_lg = attn_ctx.enter_context(
            tc.tile_pool(name="ps_lg", bufs=2, space="PSUM"))
        ps_ctx = attn_ctx.enter_context(
            tc.tile_pool(name="ps_ctx", bufs=2, space="PSUM"))
        for pair in range(NJ):
            for Hh in range(2):
                if is_causal:
                    i_list = [i for i in range(NT)
                              if any(status[i][j] != "skip"
                                     for j in range(Hh * 4, Hh * 4 + 4))]
                else:
                    i_list = list(range(NT))
                psc = ps_ctx.tile([P, 2, 512], f32, tag="ctx", name="psc")
                for idx, i in enumerate(i_list):
                    s0 = max(0, i * P - Hh * 512) if is_causal else 0
                    W = 512 - s0
                    # the two heads of the pair sit in array row-groups
                    # 0-63 / 64-127 -> adjacent K=64 matmuls run concurrently;
                    # both land in one 2-bank PSUM tile for a single exp
                    plg = ps_lg.tile([P, 2, 512], f32, tag="lg")
                    for sub in range(2):
                        pb = sub * DEP
                        nc.tensor.matmul(
                            plg[:, sub, s0:512],
                            kT[pb:pb + DEP, pair, i * P:(i + 1) * P],
                            qT[pb:pb + DEP, pair,
                               Hh * 512 + s0:(Hh + 1) * 512],
                            start=True, stop=True,
                        )
                    et = epool.tile([P, 2, 512], bf16, tag="e")
                    nc.scalar.activation(
                        et[:, :, s0:512], plg[:, :, s0:512], AF.Exp,
                        scale=0.125,
                    )
                    for j in range((Hh * 512 + s0) // P, Hh * 4 + 4):
                        if status[i][j] == "full":
                            continue
                        c = j * P - Hh * 512
                        mt = mtiles.get((i, j))
                        if mt is None:  # 'skip' inside computed range
                            nc.vector.memset(et[:, :, c:c + P], 0.0)
                        else:
                            nc.vector.tensor_mul(
                                et[:, :, c:c + P], et[:, :, c:c + P],
                                _rep2(mt),
                            )
                    for sub in range(2):
                        h = pair * 2 + sub
                        nc.tensor.matmul(
                            psc[0:DEP + 1, sub, s0:512],
                            vps[i][:, h, :],
                            et[:, sub, s0:512],
                            start=(idx == 0), stop=(idx == len(i_list) - 1),
                        )
                # evict unnormalized ctx (cross-partition bf16 copies) and
                # bounce the raw den rows to DRAM (PSUM is DMA-unreachable,
                # so they hop via a tiny SBUF tile)
                g = pair * 2 + Hh
                cols = slice(Hh * 512, (Hh + 1) * 512)
                nc.vector.tensor_copy(ctxT[0:DEP, pair, cols],
                                      psc[0:DEP, 0, :])
                nc.vector.tensor_copy(ctxT[DEP:P, pair, cols],
                                      psc[0:DEP, 1, :])
                dent = bcpool.tile([1, 2, 512], bf16, tag="dent", bufs=2)
                nc.vector.tensor_copy(dent, psc[DEP:DEP + 1, :, :])
                nc.gpsimd.dma_start(den_d[g, :, :], dent)

                if g % 4 == 3:
                    # batched reciprocal for groups g-3..g: one DVE recip
                    # over 8 partition rows, then per-group partition
                    # broadcast via DMA and in-place ctx normalize. Early
                    # batches overlap the rest of attention; the last
                    # batch's tail hides under Wo's pair-ordered
                    # accumulation.
                    g0 = g - 3
                    denb = rot.tile([8, 512], bf16, tag="sub")
                    nc.sync.dma_start(denb, den_d[g0:g0 + 4, :, :])
                    recb = rot.tile([8, 512], bf16, tag="sub")
                    _dve_recip(nc, recb, denb)
                    nc.sync.dma_start(rec_d[g0:g0 + 4, :, :], recb)
                    for gg in range(g0, g0 + 4):
                        pr, hh = gg // 2, gg % 2
                        ccols = slice(hh * 512, (hh + 1) * 512)
                        bcs = bcpool.tile([P, 512], bf16, tag="bc", bufs=2)
                        nc.sync.dma_start(
                            bcs, bcast2_ap(rec_d[gg, :, :], DEP))
                        # normalize on GpSimd (all-SBUF bf16) to keep the
                        # DVE FIFO free for the eviction chain
                        nc.gpsimd.tensor_mul(
                            ctxT[:, pr, ccols], ctxT[:, pr, ccols], bcs)

        attn_ctx.close()
        ps_mm = ctx.enter_context(
            tc.tile_pool(name="ps_mm2", bufs=2, space="PSUM"))

        # ---- Wo + bias + residual -> o1f (bf16), then LN1 -----------------
        o1f = bigp.tile([P, NJ, S], bf16, tag="o1f")
        for n in range(NJ):
            for Hh in range(2):
                cols = slice(Hh * 512, (Hh + 1) * 512)
                pw = ps_mm.tile([P, 512], f32, tag="mm")
                for pair in range(NJ):
                    nc.tensor.matmul(
                        pw,
                        wo_sb[:, pair, n * P:(n + 1) * P],
                        ctxT[:, pair, cols],
                        start=(pair == 0), stop=(pair == NJ - 1),
                    )
                nc.vector.scalar_tensor_tensor(
                    out=o1f[:, n, cols], in0=pw, scalar=boT[:, n:n + 1],
                    in1=xTb[:, n, cols], op0=ALU.add, op1=ALU.add,
                )

        # ---- LN1 (transposed; sums read bf16 o1f directly) ----------------
        ln_ctx = ExitStack()
        ps_ln = ln_ctx.enter_context(
            tc.tile_pool(name="ps_ln", bufs=1, space="PSUM"))
        ps_s = [ps_ln.tile([P, 512], f32, tag=f"lns{Hh}", name=f"lns{Hh}")
                for Hh in range(2)]
        ps_q = [ps_ln.tile([P, 512], f32, tag=f"lnq{Hh}", name=f"lnq{Hh}")
                for Hh in range(2)]
        o1n = sing.tile([P, NJ, S], bf16, tag="ko")
        for Hh in range(2):
            cols = slice(Hh * 512, (Hh + 1) * 512)
            sq = rot.tile([P, NJ, 512], bf16, tag="sq", bufs=1)
            nc.scalar.activation(sq, o1f[:, :, cols], AF.Square)
            for n in range(NJ):
                nc.tensor.matmul(ps_s[Hh], ones_bf, o1f[:, n, cols],
                                 start=(n == 0), stop=(n == NJ - 1))
                nc.tensor.matmul(ps_q[Hh], ones_bf, sq[:, n, :],
                                 start=(n == 0), stop=(n == NJ - 1))
            mean = statp.tile([P, 512], f32, tag="mean")
            nc.vector.tensor_scalar_mul(mean, ps_s[Hh], 1.0 / D)
            m2 = rot.tile([P, 512], f32, tag="sub")
            nc.vector.tensor_mul(m2, mean, mean)
            var = statp.tile([P, 512], f32, tag="var")
            nc.vector.scalar_tensor_tensor(
                out=var, in0=ps_q[Hh], scalar=1.0 / D, in1=m2,
                op0=ALU.mult, op1=ALU.subtract,
            )
            nc.scalar.activation(var, var, AF.Sqrt, bias=eps_t)
            nc.vector.reciprocal(var, var)  # rstd
            for j in range(NJ):
                sl = o1f[:, j, cols]
                sub = rot.tile([P, 512], f32, tag="sub")
                nc.vector.tensor_sub(sub, sl, mean)
                nc.vector.tensor_mul(sub, sub, var)
                nc.vector.tensor_scalar(
                    out=o1n[:, j, cols], in0=sub,
                    scalar1=g1T[:, j:j + 1], scalar2=be1T[:, j:j + 1],
                    op0=ALU.mult, op1=ALU.add,
                )

        # ---- FFN + LN3 (FFN2 swapped -> natural layout) -------------------
        ln_ctx.close()

        b2_b = sing.tile([P, D], f32, tag="natb")
        nc.gpsimd.dma_start(b2_b, bcast_ap(b2_d[:], P))
        g3_b = sing.tile([P, D], f32, tag="g3_b")
        nc.gpsimd.dma_start(g3_b, bcast_ap(g3_d[:], P))
        be3_b = sing.tile([P, D], f32, tag="be3_b")
        nc.gpsimd.dma_start(be3_b, bcast_ap(be3_d[:], P))
        nat_ctx = ExitStack()
        ps_nat = nat_ctx.enter_context(
            tc.tile_pool(name="ps_nat", bufs=2, space="PSUM"))
        # W2 fully resident: after Wo the xTb / vps / ctxT / wo_sb slots are
        # dead — exactly 64KB/partition — so the whole of W2 moves into them
        # during LN1/FFN1 and FFN2 streams nothing.
        w2h = bigp.tile([P, 8, D], bf16, tag="bigx")
        nc.sync.dma_start(w2h, w2_v[:, 0:8, :])
        w2x = []
        for i in range(8):
            t = sing.tile([P, D], bf16, tag=f"vp{i}", name=f"w2x{i}")
            nc.gpsimd.dma_start(t, w2_v[:, 8 + i, :])
            w2x.append(t)
        w2c = sing.tile([P, 8, D], bf16, tag="ctxT")
        nc.sync.dma_start(w2c, w2_v[:, 16:24, :])
        w2w = sing.tile([P, 8, D], bf16, tag="wo_sb")
        nc.gpsimd.dma_start(w2w, w2_v[:, 24:32, :])

        def w2_res(m):
            if m < 8:
                return w2h[:, m, :]
            if m < 16:
                return w2x[m - 8]
            if m < 24:
                return w2c[:, m - 16, :]
            return w2w[:, m - 24, :]

        for Hh in range(2):
            h1 = bigp.tile([P, NM, 512], bf16, tag="o1h")
            for nb in range(8):
                wt = wpool.tile([P, NJ, 512], bf16, tag="wbig")
                nc.sync.dma_start(wt, w1_v[:, :, nb * 512:(nb + 1) * 512])
                for mloc in range(4):
                    m = nb * 4 + mloc
                    psum = ps_mm.tile([P, 512], f32, tag="mm")
                    for j in range(NJ):
                        nc.tensor.matmul(
                            psum,
                            wt[:, j, mloc * P:(mloc + 1) * P],
                            o1n[:, j, Hh * 512:(Hh + 1) * 512],
                            start=(j == 0), stop=(j == NJ - 1),
                        )
                    nc.scalar.activation(
                        h1[:, m, :], psum, AF.Relu, bias=b1T[:, m:m + 1]
                    )
            for tp in range(2):
                pnats = [ps_nat.tile([P, D], f32, tag="nat", name=f"nat{Hh}{tp}{ti}")
                         for ti in range(2)]
                for m in range(NM):
                    w2t = w2_res(m)
                    for ti in range(2):
                        tloc = tp * 2 + ti
                        for half in range(2):
                            nc.tensor.matmul(
                                pnats[ti][:, half * 512:(half + 1) * 512],
                                h1[:, m, tloc * P:(tloc + 1) * P],
                                w2t[:, half * 512:(half + 1) * 512],
                                start=(m == 0), stop=(m == NM - 1),
                            )
                for ti in range(2):
                    t = Hh * 4 + tp * 2 + ti
                    pnat = pnats[ti]
                    onat = onatp.tile([P, D], f32, tag="onat")
                    nc.scalar.activation(onat, pnat, AF.Copy)  # frees psum
                    nc.vector.tensor_add(onat, onat, b2_b)
                    stats = statp.tile([P, 2, 6], f32, tag="bnst")
                    nc.vector.bn_stats(stats[:, 0, :], onat[:, 0:512])
                    nc.vector.bn_stats(stats[:, 1, :], onat[:, 512:1024])
                    mv = statp.tile([P, 2], f32, tag="bnmv")
                    nc.vector.bn_aggr(mv, stats)
                    rs = statp.tile([P, 1], f32, tag="bnrs")
                    nc.scalar.activation(rs, mv[:, 1:2], AF.Sqrt, bias=eps_t)
                    nc.vector.reciprocal(rs, rs)
                    nc.vector.tensor_scalar(
                        out=onat, in0=onat, scalar1=mv[:, 0:1], scalar2=rs,
                        op0=ALU.subtract, op1=ALU.mult,
                    )
                    nc.vector.tensor_mul(onat, onat, g3_b)
                    obf = rot.tile([P, D], bf16, tag="sub")
                    nc.vector.tensor_add(obf, onat, be3_b)
                    nc.sync.dma_start(out_d[t * P:(t + 1) * P, :], obf)
        nat_ctx.close()

    return nc


_BUILD_CACHE = {}


def _get_nc(mask_qk):
    key = mask_qk.tobytes()
    if key not in _BUILD_CACHE:
        status, is_causal = _block_plan(mask_qk)
        nc = build_nc(status, is_causal)
        legalize_waits(nc)
        _BUILD_CACHE[key] = nc
    return _BUILD_CACHE[key]


def kernel(x, look_ahead_mask, wq, bq, wk, bk, wv, bv, wo, bo,
           w1, b1, w2, b2, ln1_g, ln1_b, ln3_g, ln3_b):
    global LAST_EXEC_NS
    x = np.asarray(x, dtype=np.float32)
    B = x.shape[0]
    mask = np.asarray(look_ahead_mask, dtype=np.float32)[0, 0]
    mask_qk = mask == 0.0  # True where key visible to query
    maskm = np.ascontiguousarray(mask_qk.T).astype(ml_dtypes.bfloat16)

    nc = _get_nc(mask_qk)

    bf = ml_dtypes.bfloat16
    shared = {
        "wq": np.ascontiguousarray(wq).astype(bf),
        "wk": np.ascontiguousarray(wk).astype(bf),
        "wv": np.ascontiguousarray(wv).astype(bf),
        "wo": np.ascontiguousarray(wo).astype(bf),
        "w1": np.ascontiguousarray(w1).astype(bf),
        "w2": np.ascontiguousarray(w2).astype(bf),
        "bq": np.asarray(bq, np.float32), "bk": np.asarray(bk, np.float32),
        "bv": np.asarray(bv, np.float32), "bo": np.asarray(bo, np.float32),
        "b1": np.asarray(b1, np.float32), "b2": np.asarray(b2, np.float32),
        "g1": np.asarray(ln1_g, np.float32),
        "be1": np.asarray(ln1_b, np.float32),
        "g3": np.asarray(ln3_g, np.float32),
        "be3": np.asarray(ln3_b, np.float32),
        "maskm": maskm,
    }
    in_maps = []
    for c in range(8):
        m = dict(shared)
        m["xTb"] = np.ascontiguousarray(x[c % B].T).astype(bf)
        in_maps.append(m)

    kwargs = {}
    if TRACE:
        kwargs = {"trace": True, "tmpdir": TRACE_DIR}
    res = run_bass_kernel_spmd(nc, in_maps, list(range(8)), **kwargs)
    LAST_EXEC_NS = res.exec_time_ns
    out = np.stack([res.results[c]["out"] for c in range(B)], axis=0)
    return out.astype(np.float32)
